# revision 61
# baseline (speedup 1.0000x reference)
"""Multi-head self-attention (RoPE, causal softmax) — Trainium2 Bass kernel.

Sharding over 8 NeuronCores: batch (2) x head-groups (16 heads / 4 groups).
Each core handles one batch element and 4 heads. Single software-pipelined
phase, streamed per query/key column block j (512 positions):

  - x / weights / cos/sin stream in as few batched multi-tile DMAs (the
    565ns-per-DMA descriptor-issue rate, not bytes, paces the prologue);
    x and the weights travel in bf16, halving prologue bytes; the first
    projection matmul starts ~3us in instead of ~36us
  - q/k projections for chunk j (head pairs stacked on partitions), RoPE
    via DVE stream_shuffle: head dims are host-permuted into a 16-deep
    interleave (x1/x2 component blocks of 16 rows inside each 32-partition
    quadrant) so the rotation partner is a within-quadrant 16-row swap —
    no second "rotated weights" matmul stream on the PE
  - v projection for key blocks 4j..4j+3 into v_aug (extra ones-column so
    the softmax denominator falls out of the attn@v matmul, PSUM row 64)
  - causal attention for column j, both head pairs: scoresT[m,n] blocks
    on PE writing a paired PSUM tile [128, 2, w], exp on ACT (scale=1/8
    fused) over both heads at once, causal mask via one gpsimd
    affine_select (pattern [[0,2],[1,w]]), attn@v per head; diagonal
    (masked) blocks are computed first so the end-of-loop drain consumes
    mask-free full blocks; scores run LAG blocks ahead of their attn@v
    consumers so the in-order PE never stalls on the exp->mask chain
  - per-head normalize, split in two stages: (A) DVE reciprocals of the
    denominator rows right after the i-loop, (B) ones-column PE broadcast
    matmul + drain + multiplies, deferred until fresh matmul work (next
    chunk's projections, wo) covers the latency; the odd head's rows reach
    partitions 64-127 via a gpsimd cross-partition copy (gpsimd may not
    touch PSUM, DVE may not cross partitions; this split satisfies both)
  - output projection wo(j-1) is emitted between attn pair 0 and pair 1 of
    chunk j; the final wo borrows the idle score-pool PSUM to run two
    output chunks in flight; partials are written to DRAM in bf16 via two
    batched half-column DMAs per chunk (host accumulates in fp32)

Attention matmuls run in float32r (full rate on the PE array); the x/W
projections run in bf16. Measured rel err ~3.7e-3 vs the fp32 reference.
"""

from contextlib import ExitStack

import ml_dtypes
import numpy as np

import concourse.bass as bass
import concourse.bacc as bacc
import concourse.tile as tile
from concourse import mybir
from concourse.bass_utils import run_bass_kernel_spmd

# problem shape (hardcoded: graded standalone)
B, S, D, H, DK = 2, 2048, 1024, 16, 64
NCORES = 8
GROUPS = NCORES // B  # 4 head-groups (cores) per batch element
NH = H // GROUPS      # 4 heads per core
ROPE_THETA = 10000.0

P = 128
NQ = 512              # query-block (matmul moving free dim)
NT = S // NQ          # 4 query blocks
NM = S // P           # 16 key blocks
KT = D // P           # 8 contraction tiles for the x-projections
OT = NH * DK // P     # 2 stacked head-pair tiles for q/k
LAG = 7               # score blocks emitted ahead of attn@v consumers

F32 = mybir.dt.float32
F32R = mybir.dt.float32r
BF16 = mybir.dt.bfloat16

# swap the two 16-row component blocks inside each 32-partition quadrant
SHUF_MASK = list(range(16, 32)) + list(range(0, 16))


def build_nc():
    nc = bacc.Bacc("TRN2", target_bir_lowering=False, debug=False)

    xt = nc.dram_tensor("xt", [D, S], BF16, kind="ExternalInput")        # x[b].T
    wq = nc.dram_tensor("wq", [D, NH * DK], BF16, kind="ExternalInput")  # perm'd, T
    wk = nc.dram_tensor("wk", [D, NH * DK], BF16, kind="ExternalInput")
    wv = nc.dram_tensor("wv", [D, NH * DK], BF16, kind="ExternalInput")
    wo = nc.dram_tensor("wo", [OT, P, D], BF16, kind="ExternalInput")   # pair-stacked
    cosr = nc.dram_tensor("cosr", [P, S], F32, kind="ExternalInput")
    sinr = nc.dram_tensor("sinr", [P, S], F32, kind="ExternalInput")    # sign-folded
    yt = nc.dram_tensor("yt", [D, S], BF16, kind="ExternalOutput")      # partial y.T
    ones_dram = nc.dram_tensor("ones", [P, DK], F32R, kind="ExternalInput")

    with tile.TileContext(nc) as tc, ExitStack() as ctx, \
            nc.allow_low_precision(reason="float32r matmul inputs and bf16 partial outputs are within tolerance"):
        # ---- persistent SBUF (whole kernel) ----
        persist = ctx.enter_context(tc.tile_pool(name="persist", bufs=1))
        cos_sb = persist.tile([P, S], F32, name="cos_sb")
        sin_sb = persist.tile([P, S], F32, name="sin_sb")
        ones_sb = persist.tile([P, DK], F32R, name="ones_sb")
        qtr_sb = persist.tile([P, OT, S], F32R, name="qtr_sb")    # rope(q)^T pairs
        ktr_sb = persist.tile([P, OT, S], F32R, name="ktr_sb")
        vaug_sb = persist.tile([P, NM, NH, DK + 1], F32R, name="vaug_sb")
        outt_sb = persist.tile([P, OT, S], BF16, name="outt_sb")  # out^T pairs
        wq_sb = persist.tile([P, KT, NH * DK], BF16, name="wq_sb")
        wk_sb = persist.tile([P, KT, NH * DK], BF16, name="wk_sb")
        wv_sb = persist.tile([P, KT, NH * DK], BF16, name="wv_sb")
        wo_sb = persist.tile([P, OT, D], BF16, name="wo_sb")
        xt_sb = persist.tile([P, KT, 2, NQ], BF16, name="xt_sb")  # j-chunk double buffer

        # prologue DMAs: batched multi-tile transfers — the SP queue's
        # 565ns/DMA issue rate, not bytes, paces the prologue, so one
        # 3-dim-AP DMA per tensor beats eight per-tile DMAs. xt rides the
        # scalar queue so both descriptor generators run in parallel.
        nc.sync.dma_start(wq_sb[:, 0:2, :],
                          wq[0:2 * P, :].rearrange("(k p) c -> p k c", k=2))
        nc.sync.dma_start(wk_sb[:, 0:2, :],
                          wk[0:2 * P, :].rearrange("(k p) c -> p k c", k=2))
        nc.sync.dma_start(wq_sb[:, 2:KT // 2, :],
                          wq[2 * P:D // 2, :].rearrange("(k p) c -> p k c", k=2))
        nc.sync.dma_start(wk_sb[:, 2:KT // 2, :],
                          wk[2 * P:D // 2, :].rearrange("(k p) c -> p k c", k=2))
        nc.sync.dma_start(wq_sb[:, KT // 2:, :],
                          wq[D // 2:, :].rearrange("(k p) c -> p k c", k=KT // 2))
        nc.sync.dma_start(wk_sb[:, KT // 2:, :],
                          wk[D // 2:, :].rearrange("(k p) c -> p k c", k=KT // 2))
        nc.scalar.dma_start(xt_sb[:, 0:2, 0, :],
                            xt[0:2 * P, 0:NQ].rearrange("(k p) s -> p k s", k=2))
        nc.scalar.dma_start(xt_sb[:, 2:KT // 2, 0, :],
                            xt[2 * P:D // 2, 0:NQ].rearrange("(k p) s -> p k s", k=2))
        nc.scalar.dma_start(xt_sb[:, KT // 2:, 0, :],
                            xt[D // 2:, 0:NQ].rearrange("(k p) s -> p k s", k=KT // 2))
        nc.sync.dma_start(cos_sb[:, 0:NQ], cosr[:, 0:NQ])
        nc.sync.dma_start(sin_sb[:, 0:NQ], sinr[:, 0:NQ])
        nc.sync.dma_start(wv_sb[:, :, :], wv[:, :].rearrange("(k p) c -> p k c", k=KT))
        # ones for the denominator broadcast; the scattered v_aug ones
        # column is filled by a cheap DVE copy (a strided DMA costs ~3.6us)
        nc.sync.dma_start(ones_sb[:], ones_dram[:, :])
        nc.vector.tensor_copy(vaug_sb[:, :, :, DK],
                              ones_sb[:].rearrange("p (a b) -> p a b", a=NM))

        pools = (
            tc.tile_pool(name="pp_ps", bufs=2, space="PSUM"),      # proj/v/bcast/wo
            tc.tile_pool(name="score_ps", bufs=2, space="PSUM"),   # paired scores
            tc.tile_pool(name="oaug_ps", bufs=2, space="PSUM"),
            tc.tile_pool(name="rope_sb", bufs=9),
            tc.tile_pool(name="exp_sb", bufs=10),
            tc.tile_pool(name="norm_sb", bufs=12),
            tc.tile_pool(name="fin_sb", bufs=2),
        )
        with pools[0] as pp_ps, pools[1] as score_ps, pools[2] as oaug_ps, \
                pools[3] as rope_sb, pools[4] as exp_sb, pools[5] as norm_sb, \
                pools[6] as fin_sb:

            def _proj_qk(t, j):
                """q+k projection + rope for head pair t, chunk j.

                Pair 0 (emitted while the score pool is idle) interleaves the
                q/k k-loops in one paired score-pool tile — matching the
                prologue DMA triplet pacing. Pair 1 is emitted during attn0's
                score cycling, so it uses sequential k-loops on single-bank
                pp tiles instead of stealing a score buffer."""
                csl = slice(j * NQ, (j + 1) * NQ)
                if t == 0:
                    # interleave the q/k k-loops in one paired score-pool
                    # tile (idle during projections), matching DMA pacing
                    ps2 = score_ps.tile([P, 2, NQ], F32, name="proj_ps", tag="sc")
                    pss = (ps2[:, 0, :], ps2[:, 1, :])
                    for k in range(KT):
                        for qk, w_sb in ((0, wq_sb), (1, wk_sb)):
                            nc.tensor.matmul(
                                pss[qk],
                                lhsT=w_sb[:, k, t * P:(t + 1) * P],
                                rhs=xt_sb[:, k, j % 2, :],
                                start=(k == 0), stop=(k == KT - 1))
                else:
                    psq = pp_ps.tile([P, NQ], F32, name="projq_ps", tag="pp")
                    psk = pp_ps.tile([P, NQ], F32, name="projk_ps", tag="pp")
                    pss = (psq[:], psk[:])
                    for qk, w_sb in ((0, wq_sb), (1, wk_sb)):
                        for k in range(KT):
                            nc.tensor.matmul(
                                pss[qk],
                                lhsT=w_sb[:, k, t * P:(t + 1) * P],
                                rhs=xt_sb[:, k, j % 2, :],
                                start=(k == 0), stop=(k == KT - 1))
                for qk, dst in ((0, qtr_sb), (1, ktr_sb)):
                    ps = pss[qk]
                    shuf = rope_sb.tile([P, NQ], F32, name="shuf", tag="rope")
                    t1 = rope_sb.tile([P, NQ], F32, name="rope_a", tag="rope")
                    t2 = rope_sb.tile([P, NQ], F32, name="rope_b", tag="rope")
                    nc.vector.stream_shuffle(shuf[:], ps, SHUF_MASK)
                    nc.vector.tensor_mul(t1[:], ps, cos_sb[:, csl])
                    nc.gpsimd.tensor_mul(t2[:], shuf[:], sin_sb[:, csl])
                    nc.vector.tensor_add(dst[:, t, csl], t1[:], t2[:])

            def _proj_v(st):
                psv = pp_ps.tile([P, NH * DK], F32, name="projv_ps", tag="pp")
                for k in range(KT):
                    nc.tensor.matmul(
                        psv[:],
                        lhsT=xt_sb[:, k, (st // 4) % 2, (st % 4) * P:(st % 4 + 1) * P],
                        rhs=wv_sb[:, k, :],
                        start=(k == 0), stop=(k == KT - 1))
                dst = vaug_sb[:, st, :, 0:DK]
                src = psv[:].rearrange("p (h c) -> p h c", h=NH)
                nc.scalar.copy(dst, src)

            def _attn(pr, j):
                """Causal attention for head pair pr, query chunk j."""
                imax = 4 * j + 3
                nsl = slice(j * NQ, (j + 1) * NQ)
                oa = [oaug_ps.tile([DK + 1, NQ], F32, name="oaug") for _ in range(2)]

                def _scores(i, start, stop):
                    # diagonal blocks (offset d = i - 4j): columns below
                    # 128*d are fully causal-masked — compute only [c0:512)
                    d = max(i - 4 * j, 0)
                    c0 = P * d
                    w = NQ - c0
                    sc = score_ps.tile([P, 2, NQ], F32, name="score", tag="sc")
                    for hh in range(2):
                        hb = hh * DK
                        nc.tensor.matmul(
                            sc[:, hh, 0:w],
                            lhsT=ktr_sb[hb:hb + DK, pr, i * P:(i + 1) * P],
                            rhs=qtr_sb[hb:hb + DK, pr,
                                       j * NQ + c0:(j + 1) * NQ],
                            start=True, stop=True)
                    eb = exp_sb.tile([P, 2, NQ], F32R, name="expblk")
                    nc.scalar.activation(
                        eb[:, :, 0:w], sc[:, :, 0:w],
                        mybir.ActivationFunctionType.Exp,
                        scale=float(1.0 / np.sqrt(DK)))
                    if i >= 4 * j:   # diagonal block: causal mask, both heads
                        nc.gpsimd.affine_select(
                            out=eb[:, :, 0:w], in_=eb[:, :, 0:w],
                            compare_op=mybir.AluOpType.is_ge,
                            fill=0.0,
                            base=0,
                            channel_multiplier=-1,
                            pattern=[[0, 2], [1, w]])
                    return (eb, c0, w, start, stop)

                def _attnv(i, blk):
                    eb, c0, w, start, stop = blk
                    for hh in range(2):
                        nc.tensor.matmul(
                            oa[hh][:, c0:NQ],
                            lhsT=vaug_sb[:, i, 2 * pr + hh, :],
                            rhs=eb[:, hh, 0:w],
                            start=start, stop=stop)

                # diagonal blocks first: their exp->mask chains are the
                # longest, so the drain attn@v's at the end consume mask-free
                # full blocks instead of waiting on fresh masks
                i_order = list(range(4 * j, imax + 1)) + list(range(0, 4 * j))
                pending = {}
                for pos, i in enumerate(i_order):
                    pending[i] = _scores(i, start=(pos == 0),
                                         stop=(pos == imax))
                    if pos >= LAG:
                        ii = i_order[pos - LAG]
                        _attnv(ii, pending.pop(ii))
                for ii in i_order[max(0, imax + 1 - LAG):]:
                    _attnv(ii, pending.pop(ii))

                # normalize part A: reciprocals of the denominator rows (row
                # 64, the ones column of v) — emitted now so they are done on
                # DVE well before part B's broadcast matmuls need them
                rcs = []
                for hh in (1, 0):
                    rc = norm_sb.tile([DK + 1, NQ], F32R, name="recip", tag="nrm")
                    nc.vector.reciprocal(rc[DK:DK + 1, :], oa[hh][DK:DK + 1, :])
                    rcs.append((hh, rc))
                return oa, rcs

            def _norm_b(pr, j, oa, rcs, tail=False):
                """Normalize part B: broadcast + multiply into outt."""
                nsl = slice(j * NQ, (j + 1) * NQ)
                bss = []
                for hh, rc in rcs:
                    bc = pp_ps.tile([DK, NQ], F32, name="bcast_ps", tag="pp")
                    nc.tensor.matmul(
                        bc[:],
                        lhsT=ones_sb[DK:DK + 1, :],
                        rhs=rc[DK:DK + 1, :],
                        start=True, stop=True)
                    bs = norm_sb.tile([DK, NQ], F32, name="bcast_sb", tag="nrm")
                    if tail:
                        # final chain: ACT is idle, shorten the DVE span
                        nc.scalar.copy(bs[:], bc[:])
                    else:
                        nc.vector.tensor_copy(bs[:], bc[:])
                    bss.append((hh, bs))
                for hh, bs in bss:
                    if hh == 0:
                        nc.vector.tensor_mul(
                            outt_sb[0:DK, pr, nsl], oa[hh][0:DK, :], bs[:])
                    else:
                        # odd head: normalize at base 0 (DVE cannot cross
                        # partitions), then gpsimd places rows 64-127
                        ot_tmp = norm_sb.tile([DK, NQ], BF16, name="ot_tmp",
                                              tag="nrm")
                        nc.vector.tensor_mul(ot_tmp[:], oa[hh][0:DK, :], bs[:])
                        nc.gpsimd.tensor_copy(outt_sb[DK:P, pr, nsl], ot_tmp[:])

            def _wo_col(j, tail=False):
                nsl = slice(j * NQ, (j + 1) * NQ)
                half = D // P // 2
                fin = fin_sb.tile([P, D // P, NQ], BF16, name="fin")
                for ot in range(D // P):
                    if tail and ot % 2 == 1:
                        # attention is over: borrow the idle score pool so
                        # two output chunks are in flight
                        ps2 = score_ps.tile([P, 2, NQ], F32, name="wo_ps2",
                                            tag="sc")
                        ps = ps2[:, 0, :]
                    else:
                        ps = pp_ps.tile([P, NQ], F32, name="wo_ps", tag="pp")
                    for t in range(OT):
                        nc.tensor.matmul(
                            ps[:],
                            lhsT=wo_sb[:, t, ot * P:(ot + 1) * P],
                            rhs=outt_sb[:, t, nsl],
                            start=(t == 0), stop=(t == OT - 1))
                    # alternate engines so consecutive PSUM drains pipeline
                    if ot % 2 == 0:
                        nc.vector.tensor_copy(fin[:, ot, :], ps[:])
                    else:
                        nc.scalar.copy(fin[:, ot, :], ps[:])
                    if ot == half - 1:
                        # batched half-column writeback: one DMA issue beats
                        # four 700ns-spaced per-chunk issues
                        nc.sync.dma_start(
                            yt[0:half * P, nsl].rearrange(
                                "(a p) s -> p a s", a=half),
                            fin[:, 0:half, :])
                    elif ot == D // P - 1:
                        nc.sync.dma_start(
                            yt[half * P:, nsl].rearrange(
                                "(a p) s -> p a s", a=half),
                            fin[:, half:, :])

            for j in range(NT):
                # prefetch chunk j+1 while computing chunk j
                if j + 1 < NT:
                    jsl = slice((j + 1) * NQ, (j + 2) * NQ)
                    nc.sync.dma_start(
                        xt_sb[:, :, (j + 1) % 2, :],
                        xt[:, jsl].rearrange("(k p) s -> p k s", k=KT))
                    nc.sync.dma_start(cos_sb[:, jsl], cosr[:, jsl])
                    nc.sync.dma_start(sin_sb[:, jsl], sinr[:, jsl])
                if j == 0:
                    nc.sync.dma_start(wo_sb[:, :, :],
                                      wo[:, :, :].rearrange("t p d -> p t d"))
                # pair 0's q/k first: its rope chain gates the first scores;
                # pair 1's projection is deferred past attn0 so its rope
                # chain (DVE/Pool) doesn't queue ahead of attn0's masks
                _proj_qk(0, j)
                for st in range(4 * j, 4 * j + 4):
                    _proj_v(st)
                if j > 0:
                    _norm_b(1, j - 1, *saved)   # recips long done: no stall
                st0 = _attn(0, j)
                _proj_qk(1, j)
                if j > 0:
                    _wo_col(j - 1)   # fills PE while pair 0's recips run
                _norm_b(0, j, *st0)
                saved = _attn(1, j)
            _norm_b(1, NT - 1, *saved, tail=True)
            _wo_col(NT - 1, tail=True)

    nc.compile()
    return nc


_NC_CACHE = {}


def _get_nc():
    if "nc" not in _NC_CACHE:
        _NC_CACHE["nc"] = build_nc()
    return _NC_CACHE["nc"]


_HALF = DK // 2
# 16-deep interleave: quadrant q holds freqs 16q..16q+15, x1 rows then x2 rows
_PERM = np.array([2 * (16 * q + i) + c
                  for q in (0, 1) for c in (0, 1) for i in range(16)])


BF16NP = ml_dtypes.bfloat16


def _prep_qk(W, heads):
    """Per-head RoPE-permuted projection weights (transposed for lhsT)."""
    Wh = W.reshape(H, DK, D)[heads][:, _PERM, :]                    # [NH, DK, D]
    return np.ascontiguousarray(Wh.reshape(NH * DK, D).T.astype(BF16NP))


def make_in_maps(x, token_positions, Wq, Wk, Wv, Wo):
    x = np.asarray(x, dtype=np.float32)
    Wq = np.asarray(Wq, dtype=np.float32)
    Wk = np.asarray(Wk, dtype=np.float32)
    Wv = np.asarray(Wv, dtype=np.float32)
    Wo = np.asarray(Wo, dtype=np.float32)
    pos = np.asarray(token_positions)

    j = np.arange(_HALF, dtype=np.float64)
    inv_freq = ROPE_THETA ** (-2.0 * j / DK)                        # [32]

    in_maps = []
    for core in range(NCORES):
        b = core // GROUPS
        g = core % GROUPS
        heads = list(range(g * NH, (g + 1) * NH))
        wq_ = _prep_qk(Wq, heads)
        wk_ = _prep_qk(Wk, heads)
        wv_ = np.ascontiguousarray(Wv.reshape(H, DK, D)[heads].reshape(NH * DK, D).T.astype(BF16NP))
        wo_ = np.ascontiguousarray(Wo.T.reshape(H, DK, D)[heads].reshape(OT, P, D).astype(BF16NP))
        ang = np.outer(inv_freq, pos[b].astype(np.float64))          # [32, S]
        cos32 = np.cos(ang)
        sin32 = np.sin(ang)
        # row layout per 64-block: [f0..15, f0..15, f16..31, f16..31]
        cos64 = np.concatenate([cos32[0:16], cos32[0:16],
                                cos32[16:32], cos32[16:32]])
        sin64 = np.concatenate([-sin32[0:16], sin32[0:16],
                                -sin32[16:32], sin32[16:32]])
        cosr = np.tile(cos64, (2, 1)).astype(np.float32)             # [128, S]
        sinr = np.tile(sin64, (2, 1)).astype(np.float32)
        in_maps.append({
            "xt": np.ascontiguousarray(x[b].T.astype(BF16NP)),
            "wq": wq_, "wk": wk_, "wv": wv_, "wo": wo_,
            "cosr": cosr, "sinr": sinr,
            "ones": np.ones((P, DK), dtype=np.float32),
        })
    return in_maps


def _gather(results):
    outs = [np.asarray(r["yt"], dtype=np.float32) for r in results]
    y = np.stack([
        sum(outs[b * GROUPS + 1: (b + 1) * GROUPS], outs[b * GROUPS]).T
        for b in range(B)
    ])
    return np.ascontiguousarray(y)


def kernel(x, token_positions, Wq, Wk, Wv, Wo):
    in_maps = make_in_maps(x, token_positions, Wq, Wk, Wv, Wo)
    res = run_bass_kernel_spmd(_get_nc(), in_maps, core_ids=list(range(NCORES)))
    return _gather(res.results)


def kernel_traced(x, token_positions, Wq, Wk, Wv, Wo, **kwargs):
    """Like kernel() but with NTFF tracing; returns (output, BassKernelResults)."""
    in_maps = make_in_maps(x, token_positions, Wq, Wk, Wv, Wo)
    res = run_bass_kernel_spmd(_get_nc(), in_maps, core_ids=list(range(NCORES)),
                               trace=True, **kwargs)
    return _gather(res.results), res


# revision 70
# speedup vs baseline: 1.0296x; 1.0296x over previous
"""Multi-head self-attention (RoPE, causal softmax) — Trainium2 Bass kernel.

Sharding over 8 NeuronCores: batch (2) x head-groups (16 heads / 4 groups).
Each core handles one batch element and 4 heads. Single software-pipelined
phase, streamed per query/key column block j (512 positions):

  - x / weights / cos/sin stream in as few batched multi-tile DMAs (the
    565ns-per-DMA descriptor-issue rate, not bytes, paces the prologue);
    x and the weights travel in bf16, halving prologue bytes; the first
    projection matmul starts ~3us in instead of ~36us
  - q/k projections for chunk j (head pairs stacked on partitions), RoPE
    via DVE stream_shuffle: head dims are host-permuted into a 16-deep
    interleave (x1/x2 component blocks of 16 rows inside each 32-partition
    quadrant) so the rotation partner is a within-quadrant 16-row swap —
    no second "rotated weights" matmul stream on the PE
  - v projection for key blocks 4j..4j+3 into v_aug (extra ones-column so
    the softmax denominator falls out of the attn@v matmul, PSUM row 64)
  - causal attention for column j, both head pairs: scoresT[m,n] blocks
    on PE writing a paired PSUM tile [128, 2, w], exp on ACT (scale=1/8
    fused) over both heads at once, causal mask via one gpsimd
    affine_select (pattern [[0,2],[1,w]]), attn@v per head; diagonal
    (masked) blocks are computed first so the end-of-loop drain consumes
    mask-free full blocks; scores run LAG blocks ahead of their attn@v
    consumers so the in-order PE never stalls on the exp->mask chain
  - per-head normalize, split in two stages: (A) DVE reciprocals of the
    denominator rows right after the i-loop, (B) ones-column PE broadcast
    matmul + drain + multiplies, deferred until fresh matmul work (next
    chunk's projections, wo) covers the latency; the odd head's rows reach
    partitions 64-127 via a gpsimd cross-partition copy (gpsimd may not
    touch PSUM, DVE may not cross partitions; this split satisfies both)
  - output projection wo(j-1) is emitted between attn pair 0 and pair 1 of
    chunk j; the final wo borrows the idle score-pool PSUM to run two
    output chunks in flight; partials are written to DRAM in bf16 via two
    batched half-column DMAs per chunk (host accumulates in fp32)

Attention matmuls run in float32r (full rate on the PE array); the x/W
projections run in bf16. Measured rel err ~3.7e-3 vs the fp32 reference.
"""

from contextlib import ExitStack

import ml_dtypes
import numpy as np

import concourse.bass as bass
import concourse.bacc as bacc
import concourse.tile as tile
from concourse import mybir
from concourse.bass_utils import run_bass_kernel_spmd

# problem shape (hardcoded: graded standalone)
B, S, D, H, DK = 2, 2048, 1024, 16, 64
NCORES = 8
GROUPS = NCORES // B  # 4 head-groups (cores) per batch element
NH = H // GROUPS      # 4 heads per core
ROPE_THETA = 10000.0

P = 128
NQ = 512              # query-block (matmul moving free dim)
NT = S // NQ          # 4 query blocks
NM = S // P           # 16 key blocks
KT = D // P           # 8 contraction tiles for the x-projections
OT = NH * DK // P     # 2 stacked head-pair tiles for q/k
LAG = 7               # score blocks emitted ahead of attn@v consumers

F32 = mybir.dt.float32
F32R = mybir.dt.float32r
BF16 = mybir.dt.bfloat16

# swap the two 16-row component blocks inside each 32-partition quadrant
SHUF_MASK = list(range(16, 32)) + list(range(0, 16))


def build_nc():
    nc = bacc.Bacc("TRN2", target_bir_lowering=False, debug=False)

    xt = nc.dram_tensor("xt", [D, S], BF16, kind="ExternalInput")        # x[b].T
    wq = nc.dram_tensor("wq", [D, NH * DK], BF16, kind="ExternalInput")  # perm'd, T
    wk = nc.dram_tensor("wk", [D, NH * DK], BF16, kind="ExternalInput")
    wv = nc.dram_tensor("wv", [D, NH * DK], BF16, kind="ExternalInput")
    wo = nc.dram_tensor("wo", [OT, P, D], BF16, kind="ExternalInput")   # pair-stacked
    cosr = nc.dram_tensor("cosr", [P, S], F32, kind="ExternalInput")
    sinr = nc.dram_tensor("sinr", [P, S], F32, kind="ExternalInput")    # sign-folded
    yt = nc.dram_tensor("yt", [D, S], BF16, kind="ExternalOutput")      # partial y.T
    ones_dram = nc.dram_tensor("ones", [P, DK], F32R, kind="ExternalInput")

    with tile.TileContext(nc) as tc, ExitStack() as ctx, \
            nc.allow_low_precision(reason="float32r matmul inputs and bf16 partial outputs are within tolerance"):
        # ---- persistent SBUF (whole kernel) ----
        persist = ctx.enter_context(tc.tile_pool(name="persist", bufs=1))
        cos_sb = persist.tile([P, S], F32, name="cos_sb")
        sin_sb = persist.tile([P, S], F32, name="sin_sb")
        ones_sb = persist.tile([P, DK], F32R, name="ones_sb")
        qtr_sb = persist.tile([P, OT, S], F32R, name="qtr_sb")    # rope(q)^T pairs
        ktr_sb = persist.tile([P, OT, S], F32R, name="ktr_sb")
        vaug_sb = persist.tile([P, NM, NH, DK + 1], F32R, name="vaug_sb")
        outt_sb = persist.tile([P, OT, S], BF16, name="outt_sb")  # out^T pairs
        wq_sb = persist.tile([P, KT, NH * DK], BF16, name="wq_sb")
        wk_sb = persist.tile([P, KT, NH * DK], BF16, name="wk_sb")
        wv_sb = persist.tile([P, KT, NH * DK], BF16, name="wv_sb")
        wo_sb = persist.tile([P, OT, D], BF16, name="wo_sb")
        xt_sb = persist.tile([P, KT, 2, NQ], BF16, name="xt_sb")  # j-chunk double buffer

        # prologue DMAs: batched multi-tile transfers — the SP queue's
        # 565ns/DMA issue rate, not bytes, paces the prologue, so one
        # 3-dim-AP DMA per tensor beats eight per-tile DMAs. xt rides the
        # scalar queue so both descriptor generators run in parallel.
        nc.sync.dma_start(wq_sb[:, 0:2, :],
                          wq[0:2 * P, :].rearrange("(k p) c -> p k c", k=2))
        nc.sync.dma_start(wk_sb[:, 0:2, :],
                          wk[0:2 * P, :].rearrange("(k p) c -> p k c", k=2))
        nc.sync.dma_start(wq_sb[:, 2:KT // 2, :],
                          wq[2 * P:D // 2, :].rearrange("(k p) c -> p k c", k=2))
        nc.sync.dma_start(wk_sb[:, 2:KT // 2, :],
                          wk[2 * P:D // 2, :].rearrange("(k p) c -> p k c", k=2))
        nc.sync.dma_start(wq_sb[:, KT // 2:, :],
                          wq[D // 2:, :].rearrange("(k p) c -> p k c", k=KT // 2))
        nc.sync.dma_start(wk_sb[:, KT // 2:, :],
                          wk[D // 2:, :].rearrange("(k p) c -> p k c", k=KT // 2))
        nc.scalar.dma_start(xt_sb[:, 0:2, 0, :],
                            xt[0:2 * P, 0:NQ].rearrange("(k p) s -> p k s", k=2))
        nc.scalar.dma_start(xt_sb[:, 2:KT // 2, 0, :],
                            xt[2 * P:D // 2, 0:NQ].rearrange("(k p) s -> p k s", k=2))
        nc.scalar.dma_start(xt_sb[:, KT // 2:, 0, :],
                            xt[D // 2:, 0:NQ].rearrange("(k p) s -> p k s", k=KT // 2))
        nc.sync.dma_start(cos_sb[:, 0:NQ], cosr[:, 0:NQ])
        nc.sync.dma_start(sin_sb[:, 0:NQ], sinr[:, 0:NQ])
        nc.sync.dma_start(wv_sb[:, :, :], wv[:, :].rearrange("(k p) c -> p k c", k=KT))
        # ones for the denominator broadcast; the scattered v_aug ones
        # column is filled by a cheap DVE copy (a strided DMA costs ~3.6us)
        nc.sync.dma_start(ones_sb[:], ones_dram[:, :])
        nc.vector.tensor_copy(vaug_sb[:, :, :, DK],
                              ones_sb[:].rearrange("p (a b) -> p a b", a=NM))

        pools = (
            tc.tile_pool(name="pp_ps", bufs=2, space="PSUM"),      # proj/v/bcast/wo
            tc.tile_pool(name="score_ps", bufs=2, space="PSUM"),   # paired scores
            tc.tile_pool(name="oaug_ps", bufs=2, space="PSUM"),
            tc.tile_pool(name="rope_sb", bufs=9),
            tc.tile_pool(name="exp_sb", bufs=10),
            tc.tile_pool(name="norm_sb", bufs=12),
            tc.tile_pool(name="fin_sb", bufs=2),
        )
        with pools[0] as pp_ps, pools[1] as score_ps, pools[2] as oaug_ps, \
                pools[3] as rope_sb, pools[4] as exp_sb, pools[5] as norm_sb, \
                pools[6] as fin_sb:

            def _proj_qk(t, j):
                """q+k projection + rope for head pair t, chunk j.

                Pair 0 (emitted while the score pool is idle) interleaves the
                q/k k-loops in one paired score-pool tile — matching the
                prologue DMA triplet pacing. Pair 1 is emitted during attn0's
                score cycling, so it uses sequential k-loops on single-bank
                pp tiles instead of stealing a score buffer."""
                csl = slice(j * NQ, (j + 1) * NQ)
                if t == 0:
                    # interleave the q/k k-loops in one paired score-pool
                    # tile (idle during projections), matching DMA pacing
                    ps2 = score_ps.tile([P, 2, NQ], F32, name="proj_ps", tag="sc")
                    pss = (ps2[:, 0, :], ps2[:, 1, :])
                    for k in range(KT):
                        for qk, w_sb in ((0, wq_sb), (1, wk_sb)):
                            nc.tensor.matmul(
                                pss[qk],
                                lhsT=w_sb[:, k, t * P:(t + 1) * P],
                                rhs=xt_sb[:, k, j % 2, :],
                                start=(k == 0), stop=(k == KT - 1))
                else:
                    psq = pp_ps.tile([P, NQ], F32, name="projq_ps", tag="pp")
                    psk = pp_ps.tile([P, NQ], F32, name="projk_ps", tag="pp")
                    pss = (psq[:], psk[:])
                    for qk, w_sb in ((0, wq_sb), (1, wk_sb)):
                        for k in range(KT):
                            nc.tensor.matmul(
                                pss[qk],
                                lhsT=w_sb[:, k, t * P:(t + 1) * P],
                                rhs=xt_sb[:, k, j % 2, :],
                                start=(k == 0), stop=(k == KT - 1))
                for qk, dst in ((0, qtr_sb), (1, ktr_sb)):
                    ps = pss[qk]
                    shuf = rope_sb.tile([P, NQ], F32, name="shuf", tag="rope")
                    t1 = rope_sb.tile([P, NQ], F32, name="rope_a", tag="rope")
                    t2 = rope_sb.tile([P, NQ], F32, name="rope_b", tag="rope")
                    nc.vector.stream_shuffle(shuf[:], ps, SHUF_MASK)
                    nc.vector.tensor_mul(t1[:], ps, cos_sb[:, csl])
                    nc.gpsimd.tensor_mul(t2[:], shuf[:], sin_sb[:, csl])
                    nc.vector.tensor_add(dst[:, t, csl], t1[:], t2[:])

            def _proj_v(st, on_dve=False):
                psv = pp_ps.tile([P, NH * DK], F32, name="projv_ps", tag="pp")
                for k in range(KT):
                    nc.tensor.matmul(
                        psv[:],
                        lhsT=xt_sb[:, k, (st // 4) % 2, (st % 4) * P:(st % 4 + 1) * P],
                        rhs=wv_sb[:, k, :],
                        start=(k == 0), stop=(k == KT - 1))
                dst = vaug_sb[:, st, :, 0:DK]
                src = psv[:].rearrange("p (h c) -> p h c", h=NH)
                if on_dve:
                    nc.vector.tensor_copy(dst, src)
                else:
                    nc.scalar.copy(dst, src)

            def _attn(pr, j):
                """Causal attention for head pair pr, query chunk j."""
                imax = 4 * j + 3
                nsl = slice(j * NQ, (j + 1) * NQ)
                oa = [oaug_ps.tile([DK + 1, NQ], F32, name="oaug") for _ in range(2)]

                def _scores(i, start, stop):
                    # diagonal blocks (offset d = i - 4j): columns below
                    # 128*d are fully causal-masked — compute only [c0:512)
                    d = max(i - 4 * j, 0)
                    c0 = P * d
                    w = NQ - c0
                    sc = score_ps.tile([P, 2, NQ], F32, name="score", tag="sc")
                    for hh in range(2):
                        hb = hh * DK
                        nc.tensor.matmul(
                            sc[:, hh, 0:w],
                            lhsT=ktr_sb[hb:hb + DK, pr, i * P:(i + 1) * P],
                            rhs=qtr_sb[hb:hb + DK, pr,
                                       j * NQ + c0:(j + 1) * NQ],
                            start=True, stop=True)
                    eb = exp_sb.tile([P, 2, NQ], F32R, name="expblk")
                    nc.scalar.activation(
                        eb[:, :, 0:w], sc[:, :, 0:w],
                        mybir.ActivationFunctionType.Exp,
                        scale=float(1.0 / np.sqrt(DK)))
                    if i >= 4 * j:   # diagonal block: causal mask, both heads
                        # masked cells satisfy col < p <= 127, so only the
                        # first 128 columns can ever be zeroed — restricting
                        # the AP there cuts the Pool op (and the chain the
                        # attn@v waits on) by 4x; later columns are already
                        # correct in SBUF
                        mw = min(w, P)
                        nc.gpsimd.affine_select(
                            out=eb[:, :, 0:mw], in_=eb[:, :, 0:mw],
                            compare_op=mybir.AluOpType.is_ge,
                            fill=0.0,
                            base=0,
                            channel_multiplier=-1,
                            pattern=[[0, 2], [1, mw]])
                    return (eb, c0, w, start, stop)

                def _attnv(i, blk):
                    eb, c0, w, start, stop = blk
                    for hh in range(2):
                        nc.tensor.matmul(
                            oa[hh][:, c0:NQ],
                            lhsT=vaug_sb[:, i, 2 * pr + hh, :],
                            rhs=eb[:, hh, 0:w],
                            start=start, stop=stop)

                # diagonal blocks first: their exp->mask chains are the
                # longest, so the drain attn@v's at the end consume mask-free
                # full blocks instead of waiting on fresh masks
                i_order = list(range(4 * j, imax + 1)) + list(range(0, 4 * j))
                pending = {}
                for pos, i in enumerate(i_order):
                    pending[i] = _scores(i, start=(pos == 0),
                                         stop=(pos == imax))
                    if pos >= LAG:
                        ii = i_order[pos - LAG]
                        _attnv(ii, pending.pop(ii))
                for ii in i_order[max(0, imax + 1 - LAG):]:
                    _attnv(ii, pending.pop(ii))

                return (oa,)

            def _recips(oa):
                rcs = []
                for hh in (1, 0):
                    rc = norm_sb.tile([DK + 1, NQ], F32R, name="recip", tag="nrm")
                    nc.vector.reciprocal(rc[DK:DK + 1, :], oa[hh][DK:DK + 1, :])
                    rcs.append((hh, rc))
                return rcs

            def _norm_b(pr, j, oa, tail=False, rcs=None):
                """Normalize: reciprocal of the denominator row (row 64, the
                ones column of v), broadcast, multiply into outt. Emitted
                behind filler matmul work so the PE never waits the chain."""
                nsl = slice(j * NQ, (j + 1) * NQ)
                if rcs is None:
                    rcs = _recips(oa)
                bss = []
                for hh, rc in rcs:
                    bc = pp_ps.tile([DK, NQ], F32, name="bcast_ps", tag="pp")
                    nc.tensor.matmul(
                        bc[:],
                        lhsT=ones_sb[DK:DK + 1, :],
                        rhs=rc[DK:DK + 1, :],
                        start=True, stop=True)
                    bs = norm_sb.tile([DK, NQ], F32, name="bcast_sb", tag="nrm")
                    if tail:
                        # final chain: ACT is idle, shorten the DVE span
                        nc.scalar.copy(bs[:], bc[:])
                    else:
                        nc.vector.tensor_copy(bs[:], bc[:])
                    bss.append((hh, bs))
                for hh, bs in bss:
                    if hh == 0:
                        nc.vector.tensor_mul(
                            outt_sb[0:DK, pr, nsl], oa[hh][0:DK, :], bs[:])
                    else:
                        # odd head: normalize at base 0 (DVE cannot cross
                        # partitions), then gpsimd places rows 64-127
                        ot_tmp = norm_sb.tile([DK, NQ], BF16, name="ot_tmp",
                                              tag="nrm")
                        nc.vector.tensor_mul(ot_tmp[:], oa[hh][0:DK, :], bs[:])
                        nc.gpsimd.tensor_copy(outt_sb[DK:P, pr, nsl], ot_tmp[:])

            def _wo_col(j, tail=False):
                nsl = slice(j * NQ, (j + 1) * NQ)
                half = D // P // 2
                fin = fin_sb.tile([P, D // P, NQ], BF16, name="fin")
                for ot in range(D // P):
                    if tail and ot % 2 == 1:
                        # attention is over: borrow the idle score pool so
                        # two output chunks are in flight
                        ps2 = score_ps.tile([P, 2, NQ], F32, name="wo_ps2",
                                            tag="sc")
                        ps = ps2[:, 0, :]
                    else:
                        ps = pp_ps.tile([P, NQ], F32, name="wo_ps", tag="pp")
                    for t in range(OT):
                        nc.tensor.matmul(
                            ps[:],
                            lhsT=wo_sb[:, t, ot * P:(ot + 1) * P],
                            rhs=outt_sb[:, t, nsl],
                            start=(t == 0), stop=(t == OT - 1))
                    # alternate engines so consecutive PSUM drains pipeline
                    if ot % 2 == 0:
                        nc.vector.tensor_copy(fin[:, ot, :], ps[:])
                    else:
                        nc.scalar.copy(fin[:, ot, :], ps[:])
                    step = 2 if tail else half
                    if (ot + 1) % step == 0:
                        # batched writeback: one DMA issue beats per-chunk
                        # 700ns-spaced issues; the tail uses quarters so the
                        # last transfer trails the last drain minimally
                        lo = ot + 1 - step
                        nc.sync.dma_start(
                            yt[lo * P:(ot + 1) * P, nsl].rearrange(
                                "(a p) s -> p a s", a=step),
                            fin[:, lo:ot + 1, :])

            for j in range(NT):
                # prefetch chunk j+1 while computing chunk j
                if j + 1 < NT:
                    jsl = slice((j + 1) * NQ, (j + 2) * NQ)
                    nc.sync.dma_start(
                        xt_sb[:, :, (j + 1) % 2, :],
                        xt[:, jsl].rearrange("(k p) s -> p k s", k=KT))
                    nc.sync.dma_start(cos_sb[:, jsl], cosr[:, jsl])
                    nc.sync.dma_start(sin_sb[:, jsl], sinr[:, jsl])
                if j == 0:
                    nc.sync.dma_start(wo_sb[:, :, :],
                                      wo[:, :, :].rearrange("t p d -> p t d"))
                # pair 0's q/k first: its rope chain gates the first scores;
                # pair 1's projection is deferred past attn0 so its rope
                # chain (DVE/Pool) doesn't queue ahead of attn0's masks
                _proj_qk(0, j)
                for st in range(4 * j, 4 * j + 4):
                    _proj_v(st)
                if j > 0:
                    _norm_b(1, j - 1, *saved)   # recips long done: no stall
                st0 = _attn(0, j)
                _proj_qk(1, j)
                if j > 0:
                    _wo_col(j - 1)   # fills PE while pair 0's recips run
                _norm_b(0, j, *st0)
                saved = _attn(1, j)
                if j == NT - 1:
                    tail_rcs = _recips(saved[0])
            _norm_b(1, NT - 1, *saved, tail=True, rcs=tail_rcs)
            _wo_col(NT - 1, tail=True)

    nc.compile()
    return nc


_NC_CACHE = {}


def _get_nc():
    if "nc" not in _NC_CACHE:
        _NC_CACHE["nc"] = build_nc()
    return _NC_CACHE["nc"]


_HALF = DK // 2
# 16-deep interleave: quadrant q holds freqs 16q..16q+15, x1 rows then x2 rows
_PERM = np.array([2 * (16 * q + i) + c
                  for q in (0, 1) for c in (0, 1) for i in range(16)])


BF16NP = ml_dtypes.bfloat16


def _prep_qk(W, heads):
    """Per-head RoPE-permuted projection weights (transposed for lhsT)."""
    Wh = W.reshape(H, DK, D)[heads][:, _PERM, :]                    # [NH, DK, D]
    return np.ascontiguousarray(Wh.reshape(NH * DK, D).T.astype(BF16NP))


def make_in_maps(x, token_positions, Wq, Wk, Wv, Wo):
    x = np.asarray(x, dtype=np.float32)
    Wq = np.asarray(Wq, dtype=np.float32)
    Wk = np.asarray(Wk, dtype=np.float32)
    Wv = np.asarray(Wv, dtype=np.float32)
    Wo = np.asarray(Wo, dtype=np.float32)
    pos = np.asarray(token_positions)

    j = np.arange(_HALF, dtype=np.float64)
    inv_freq = ROPE_THETA ** (-2.0 * j / DK)                        # [32]

    in_maps = []
    for core in range(NCORES):
        b = core // GROUPS
        g = core % GROUPS
        heads = list(range(g * NH, (g + 1) * NH))
        wq_ = _prep_qk(Wq, heads)
        wk_ = _prep_qk(Wk, heads)
        wv_ = np.ascontiguousarray(Wv.reshape(H, DK, D)[heads].reshape(NH * DK, D).T.astype(BF16NP))
        wo_ = np.ascontiguousarray(Wo.T.reshape(H, DK, D)[heads].reshape(OT, P, D).astype(BF16NP))
        ang = np.outer(inv_freq, pos[b].astype(np.float64))          # [32, S]
        cos32 = np.cos(ang)
        sin32 = np.sin(ang)
        # row layout per 64-block: [f0..15, f0..15, f16..31, f16..31]
        cos64 = np.concatenate([cos32[0:16], cos32[0:16],
                                cos32[16:32], cos32[16:32]])
        sin64 = np.concatenate([-sin32[0:16], sin32[0:16],
                                -sin32[16:32], sin32[16:32]])
        cosr = np.tile(cos64, (2, 1)).astype(np.float32)             # [128, S]
        sinr = np.tile(sin64, (2, 1)).astype(np.float32)
        in_maps.append({
            "xt": np.ascontiguousarray(x[b].T.astype(BF16NP)),
            "wq": wq_, "wk": wk_, "wv": wv_, "wo": wo_,
            "cosr": cosr, "sinr": sinr,
            "ones": np.ones((P, DK), dtype=np.float32),
        })
    return in_maps


def _gather(results):
    outs = [np.asarray(r["yt"], dtype=np.float32) for r in results]
    y = np.stack([
        sum(outs[b * GROUPS + 1: (b + 1) * GROUPS], outs[b * GROUPS]).T
        for b in range(B)
    ])
    return np.ascontiguousarray(y)


def kernel(x, token_positions, Wq, Wk, Wv, Wo):
    in_maps = make_in_maps(x, token_positions, Wq, Wk, Wv, Wo)
    res = run_bass_kernel_spmd(_get_nc(), in_maps, core_ids=list(range(NCORES)))
    return _gather(res.results)


def kernel_traced(x, token_positions, Wq, Wk, Wv, Wo, **kwargs):
    """Like kernel() but with NTFF tracing; returns (output, BassKernelResults)."""
    in_maps = make_in_maps(x, token_positions, Wq, Wk, Wv, Wo)
    res = run_bass_kernel_spmd(_get_nc(), in_maps, core_ids=list(range(NCORES)),
                               trace=True, **kwargs)
    return _gather(res.results), res


# revision 71
# speedup vs baseline: 1.0347x; 1.0050x over previous
"""Multi-head self-attention (RoPE, causal softmax) — Trainium2 Bass kernel.

Sharding over 8 NeuronCores: batch (2) x head-groups (16 heads / 4 groups).
Each core handles one batch element and 4 heads. Single software-pipelined
phase, streamed per query/key column block j (512 positions):

  - x / weights / cos/sin stream in as few batched multi-tile DMAs (the
    565ns-per-DMA descriptor-issue rate, not bytes, paces the prologue);
    x and the weights travel in bf16, halving prologue bytes; the first
    projection matmul starts ~3us in instead of ~36us
  - q/k projections for chunk j (head pairs stacked on partitions), RoPE
    via DVE stream_shuffle: head dims are host-permuted into a 16-deep
    interleave (x1/x2 component blocks of 16 rows inside each 32-partition
    quadrant) so the rotation partner is a within-quadrant 16-row swap —
    no second "rotated weights" matmul stream on the PE
  - v projection for key blocks 4j..4j+3 into v_aug (extra ones-column so
    the softmax denominator falls out of the attn@v matmul, PSUM row 64)
  - causal attention for column j, both head pairs: scoresT[m,n] blocks
    on PE writing a paired PSUM tile [128, 2, w], exp on ACT (scale=1/8
    fused) over both heads at once, causal mask via one gpsimd
    affine_select (pattern [[0,2],[1,w]]), attn@v per head; diagonal
    (masked) blocks are computed first so the end-of-loop drain consumes
    mask-free full blocks; scores run LAG blocks ahead of their attn@v
    consumers so the in-order PE never stalls on the exp->mask chain
  - per-head normalize, split in two stages: (A) DVE reciprocals of the
    denominator rows right after the i-loop, (B) ones-column PE broadcast
    matmul + drain + multiplies, deferred until fresh matmul work (next
    chunk's projections, wo) covers the latency; the odd head's rows reach
    partitions 64-127 via a gpsimd cross-partition copy (gpsimd may not
    touch PSUM, DVE may not cross partitions; this split satisfies both)
  - output projection wo(j-1) is emitted between attn pair 0 and pair 1 of
    chunk j; the final wo borrows the idle score-pool PSUM to run two
    output chunks in flight; partials are written to DRAM in bf16 via two
    batched half-column DMAs per chunk (host accumulates in fp32)

Attention matmuls run in float32r (full rate on the PE array); the x/W
projections run in bf16. Measured rel err ~3.7e-3 vs the fp32 reference.
"""

from contextlib import ExitStack

import ml_dtypes
import numpy as np

import concourse.bass as bass
import concourse.bacc as bacc
import concourse.tile as tile
from concourse import mybir
from concourse.bass_utils import run_bass_kernel_spmd

# problem shape (hardcoded: graded standalone)
B, S, D, H, DK = 2, 2048, 1024, 16, 64
NCORES = 8
GROUPS = NCORES // B  # 4 head-groups (cores) per batch element
NH = H // GROUPS      # 4 heads per core
ROPE_THETA = 10000.0

P = 128
NQ = 512              # query-block (matmul moving free dim)
NT = S // NQ          # 4 query blocks
NM = S // P           # 16 key blocks
KT = D // P           # 8 contraction tiles for the x-projections
OT = NH * DK // P     # 2 stacked head-pair tiles for q/k
LAG = 7               # score blocks emitted ahead of attn@v consumers

F32 = mybir.dt.float32
F32R = mybir.dt.float32r
BF16 = mybir.dt.bfloat16

# swap the two 16-row component blocks inside each 32-partition quadrant
SHUF_MASK = list(range(16, 32)) + list(range(0, 16))


def build_nc():
    nc = bacc.Bacc("TRN2", target_bir_lowering=False, debug=False)

    xt = nc.dram_tensor("xt", [D, S], BF16, kind="ExternalInput")        # x[b].T
    wq = nc.dram_tensor("wq", [D, NH * DK], BF16, kind="ExternalInput")  # perm'd, T
    wk = nc.dram_tensor("wk", [D, NH * DK], BF16, kind="ExternalInput")
    wv = nc.dram_tensor("wv", [D, NH * DK], BF16, kind="ExternalInput")
    wo = nc.dram_tensor("wo", [OT, P, D], BF16, kind="ExternalInput")   # pair-stacked
    cosr = nc.dram_tensor("cosr", [P, S], F32, kind="ExternalInput")
    sinr = nc.dram_tensor("sinr", [P, S], F32, kind="ExternalInput")    # sign-folded
    yt = nc.dram_tensor("yt", [D, S], BF16, kind="ExternalOutput")      # partial y.T
    ones_dram = nc.dram_tensor("ones", [P, DK], F32R, kind="ExternalInput")

    with tile.TileContext(nc) as tc, ExitStack() as ctx, \
            nc.allow_low_precision(reason="float32r matmul inputs and bf16 partial outputs are within tolerance"):
        # ---- persistent SBUF (whole kernel) ----
        persist = ctx.enter_context(tc.tile_pool(name="persist", bufs=1))
        cos_sb = persist.tile([P, S], F32, name="cos_sb")
        sin_sb = persist.tile([P, S], F32, name="sin_sb")
        ones_sb = persist.tile([P, DK], F32R, name="ones_sb")
        qtr_sb = persist.tile([P, OT, S], F32R, name="qtr_sb")    # rope(q)^T pairs
        ktr_sb = persist.tile([P, OT, S], F32R, name="ktr_sb")
        vaug_sb = persist.tile([P, NM, NH, DK + 1], F32R, name="vaug_sb")
        outt_sb = persist.tile([P, OT, S], BF16, name="outt_sb")  # out^T pairs
        wq_sb = persist.tile([P, KT, NH * DK], BF16, name="wq_sb")
        wk_sb = persist.tile([P, KT, NH * DK], BF16, name="wk_sb")
        wv_sb = persist.tile([P, KT, NH * DK], BF16, name="wv_sb")
        wo_sb = persist.tile([P, OT, D], BF16, name="wo_sb")
        xt_sb = persist.tile([P, KT, 2, NQ], BF16, name="xt_sb")  # j-chunk double buffer

        # prologue DMAs: batched multi-tile transfers — the SP queue's
        # 565ns/DMA issue rate, not bytes, paces the prologue, so one
        # 3-dim-AP DMA per tensor beats eight per-tile DMAs. xt rides the
        # scalar queue so both descriptor generators run in parallel.
        nc.sync.dma_start(wq_sb[:, 0:2, :],
                          wq[0:2 * P, :].rearrange("(k p) c -> p k c", k=2))
        nc.sync.dma_start(wq_sb[:, 2:KT // 2, :],
                          wq[2 * P:D // 2, :].rearrange("(k p) c -> p k c", k=2))
        nc.sync.dma_start(wq_sb[:, KT // 2:, :],
                          wq[D // 2:, :].rearrange("(k p) c -> p k c", k=KT // 2))
        nc.sync.dma_start(wk_sb[:, 0:KT // 2, :],
                          wk[0:D // 2, :].rearrange("(k p) c -> p k c", k=KT // 2))
        nc.sync.dma_start(wk_sb[:, KT // 2:, :],
                          wk[D // 2:, :].rearrange("(k p) c -> p k c", k=KT // 2))
        nc.scalar.dma_start(xt_sb[:, 0:2, 0, :],
                            xt[0:2 * P, 0:NQ].rearrange("(k p) s -> p k s", k=2))
        nc.scalar.dma_start(xt_sb[:, 2:KT // 2, 0, :],
                            xt[2 * P:D // 2, 0:NQ].rearrange("(k p) s -> p k s", k=2))
        nc.scalar.dma_start(xt_sb[:, KT // 2:, 0, :],
                            xt[D // 2:, 0:NQ].rearrange("(k p) s -> p k s", k=KT // 2))
        nc.sync.dma_start(cos_sb[:, 0:NQ], cosr[:, 0:NQ])
        nc.sync.dma_start(sin_sb[:, 0:NQ], sinr[:, 0:NQ])
        nc.sync.dma_start(wv_sb[:, :, :], wv[:, :].rearrange("(k p) c -> p k c", k=KT))
        # ones for the denominator broadcast; the scattered v_aug ones
        # column is filled by a cheap DVE copy (a strided DMA costs ~3.6us)
        nc.sync.dma_start(ones_sb[:], ones_dram[:, :])
        nc.vector.tensor_copy(vaug_sb[:, :, :, DK],
                              ones_sb[:].rearrange("p (a b) -> p a b", a=NM))

        pools = (
            tc.tile_pool(name="pp_ps", bufs=2, space="PSUM"),      # proj/v/bcast/wo
            tc.tile_pool(name="score_ps", bufs=2, space="PSUM"),   # paired scores
            tc.tile_pool(name="oaug_ps", bufs=2, space="PSUM"),
            tc.tile_pool(name="rope_sb", bufs=9),
            tc.tile_pool(name="exp_sb", bufs=10),
            tc.tile_pool(name="norm_sb", bufs=12),
            tc.tile_pool(name="fin_sb", bufs=2),
        )
        with pools[0] as pp_ps, pools[1] as score_ps, pools[2] as oaug_ps, \
                pools[3] as rope_sb, pools[4] as exp_sb, pools[5] as norm_sb, \
                pools[6] as fin_sb:

            def _proj_qk(t, j):
                """q+k projection + rope for head pair t, chunk j.

                Pair 0 (emitted while the score pool is idle) interleaves the
                q/k k-loops in one paired score-pool tile — matching the
                prologue DMA triplet pacing. Pair 1 is emitted during attn0's
                score cycling, so it uses sequential k-loops on single-bank
                pp tiles instead of stealing a score buffer."""
                csl = slice(j * NQ, (j + 1) * NQ)
                if t == 0:
                    # sequential q-then-k loops in one paired score-pool tile
                    # (idle during projections): the q loop depends only on
                    # wq+xt, so the prologue feed keeps it PE-bound while wk
                    # streams in behind it
                    ps2 = score_ps.tile([P, 2, NQ], F32, name="proj_ps", tag="sc")
                    pss = (ps2[:, 0, :], ps2[:, 1, :])
                    for qk, w_sb in ((0, wq_sb), (1, wk_sb)):
                        for k in range(KT):
                            nc.tensor.matmul(
                                pss[qk],
                                lhsT=w_sb[:, k, t * P:(t + 1) * P],
                                rhs=xt_sb[:, k, j % 2, :],
                                start=(k == 0), stop=(k == KT - 1))
                else:
                    psq = pp_ps.tile([P, NQ], F32, name="projq_ps", tag="pp")
                    psk = pp_ps.tile([P, NQ], F32, name="projk_ps", tag="pp")
                    pss = (psq[:], psk[:])
                    for qk, w_sb in ((0, wq_sb), (1, wk_sb)):
                        for k in range(KT):
                            nc.tensor.matmul(
                                pss[qk],
                                lhsT=w_sb[:, k, t * P:(t + 1) * P],
                                rhs=xt_sb[:, k, j % 2, :],
                                start=(k == 0), stop=(k == KT - 1))
                for qk, dst in ((0, qtr_sb), (1, ktr_sb)):
                    ps = pss[qk]
                    shuf = rope_sb.tile([P, NQ], F32, name="shuf", tag="rope")
                    t1 = rope_sb.tile([P, NQ], F32, name="rope_a", tag="rope")
                    t2 = rope_sb.tile([P, NQ], F32, name="rope_b", tag="rope")
                    nc.vector.stream_shuffle(shuf[:], ps, SHUF_MASK)
                    nc.vector.tensor_mul(t1[:], ps, cos_sb[:, csl])
                    nc.gpsimd.tensor_mul(t2[:], shuf[:], sin_sb[:, csl])
                    nc.vector.tensor_add(dst[:, t, csl], t1[:], t2[:])

            def _proj_v(st, on_dve=False):
                psv = pp_ps.tile([P, NH * DK], F32, name="projv_ps", tag="pp")
                for k in range(KT):
                    nc.tensor.matmul(
                        psv[:],
                        lhsT=xt_sb[:, k, (st // 4) % 2, (st % 4) * P:(st % 4 + 1) * P],
                        rhs=wv_sb[:, k, :],
                        start=(k == 0), stop=(k == KT - 1))
                dst = vaug_sb[:, st, :, 0:DK]
                src = psv[:].rearrange("p (h c) -> p h c", h=NH)
                if on_dve:
                    nc.vector.tensor_copy(dst, src)
                else:
                    nc.scalar.copy(dst, src)

            def _attn(pr, j):
                """Causal attention for head pair pr, query chunk j."""
                imax = 4 * j + 3
                nsl = slice(j * NQ, (j + 1) * NQ)
                oa = [oaug_ps.tile([DK + 1, NQ], F32, name="oaug") for _ in range(2)]

                def _scores(i, start, stop):
                    # diagonal blocks (offset d = i - 4j): columns below
                    # 128*d are fully causal-masked — compute only [c0:512)
                    d = max(i - 4 * j, 0)
                    c0 = P * d
                    w = NQ - c0
                    sc = score_ps.tile([P, 2, NQ], F32, name="score", tag="sc")
                    for hh in range(2):
                        hb = hh * DK
                        nc.tensor.matmul(
                            sc[:, hh, 0:w],
                            lhsT=ktr_sb[hb:hb + DK, pr, i * P:(i + 1) * P],
                            rhs=qtr_sb[hb:hb + DK, pr,
                                       j * NQ + c0:(j + 1) * NQ],
                            start=True, stop=True)
                    eb = exp_sb.tile([P, 2, NQ], F32R, name="expblk")
                    nc.scalar.activation(
                        eb[:, :, 0:w], sc[:, :, 0:w],
                        mybir.ActivationFunctionType.Exp,
                        scale=float(1.0 / np.sqrt(DK)))
                    if i >= 4 * j:   # diagonal block: causal mask, both heads
                        # masked cells satisfy col < p <= 127, so only the
                        # first 128 columns can ever be zeroed — restricting
                        # the AP there cuts the Pool op (and the chain the
                        # attn@v waits on) by 4x; later columns are already
                        # correct in SBUF
                        mw = min(w, P)
                        nc.gpsimd.affine_select(
                            out=eb[:, :, 0:mw], in_=eb[:, :, 0:mw],
                            compare_op=mybir.AluOpType.is_ge,
                            fill=0.0,
                            base=0,
                            channel_multiplier=-1,
                            pattern=[[0, 2], [1, mw]])
                    return (eb, c0, w, start, stop)

                def _attnv(i, blk):
                    eb, c0, w, start, stop = blk
                    for hh in range(2):
                        nc.tensor.matmul(
                            oa[hh][:, c0:NQ],
                            lhsT=vaug_sb[:, i, 2 * pr + hh, :],
                            rhs=eb[:, hh, 0:w],
                            start=start, stop=stop)

                # diagonal blocks first: their exp->mask chains are the
                # longest, so the drain attn@v's at the end consume mask-free
                # full blocks instead of waiting on fresh masks
                i_order = list(range(4 * j, imax + 1)) + list(range(0, 4 * j))
                pending = {}
                for pos, i in enumerate(i_order):
                    pending[i] = _scores(i, start=(pos == 0),
                                         stop=(pos == imax))
                    if pos >= LAG:
                        ii = i_order[pos - LAG]
                        _attnv(ii, pending.pop(ii))
                for ii in i_order[max(0, imax + 1 - LAG):]:
                    _attnv(ii, pending.pop(ii))

                return (oa,)

            def _recips(oa):
                rcs = []
                for hh in (1, 0):
                    rc = norm_sb.tile([DK + 1, NQ], F32R, name="recip", tag="nrm")
                    nc.vector.reciprocal(rc[DK:DK + 1, :], oa[hh][DK:DK + 1, :])
                    rcs.append((hh, rc))
                return rcs

            def _norm_b(pr, j, oa, tail=False, rcs=None):
                """Normalize: reciprocal of the denominator row (row 64, the
                ones column of v), broadcast, multiply into outt. Emitted
                behind filler matmul work so the PE never waits the chain."""
                nsl = slice(j * NQ, (j + 1) * NQ)
                if rcs is None:
                    rcs = _recips(oa)
                bss = []
                for hh, rc in rcs:
                    bc = pp_ps.tile([DK, NQ], F32, name="bcast_ps", tag="pp")
                    nc.tensor.matmul(
                        bc[:],
                        lhsT=ones_sb[DK:DK + 1, :],
                        rhs=rc[DK:DK + 1, :],
                        start=True, stop=True)
                    bs = norm_sb.tile([DK, NQ], F32, name="bcast_sb", tag="nrm")
                    if tail:
                        # final chain: ACT is idle, shorten the DVE span
                        nc.scalar.copy(bs[:], bc[:])
                    else:
                        nc.vector.tensor_copy(bs[:], bc[:])
                    bss.append((hh, bs))
                for hh, bs in bss:
                    if hh == 0:
                        nc.vector.tensor_mul(
                            outt_sb[0:DK, pr, nsl], oa[hh][0:DK, :], bs[:])
                    else:
                        # odd head: normalize at base 0 (DVE cannot cross
                        # partitions), then gpsimd places rows 64-127
                        ot_tmp = norm_sb.tile([DK, NQ], BF16, name="ot_tmp",
                                              tag="nrm")
                        nc.vector.tensor_mul(ot_tmp[:], oa[hh][0:DK, :], bs[:])
                        nc.gpsimd.tensor_copy(outt_sb[DK:P, pr, nsl], ot_tmp[:])

            def _wo_col(j, tail=False):
                nsl = slice(j * NQ, (j + 1) * NQ)
                half = D // P // 2
                fin = fin_sb.tile([P, D // P, NQ], BF16, name="fin")
                for ot in range(D // P):
                    if tail and ot % 2 == 1:
                        # attention is over: borrow the idle score pool so
                        # two output chunks are in flight
                        ps2 = score_ps.tile([P, 2, NQ], F32, name="wo_ps2",
                                            tag="sc")
                        ps = ps2[:, 0, :]
                    else:
                        ps = pp_ps.tile([P, NQ], F32, name="wo_ps", tag="pp")
                    for t in range(OT):
                        nc.tensor.matmul(
                            ps[:],
                            lhsT=wo_sb[:, t, ot * P:(ot + 1) * P],
                            rhs=outt_sb[:, t, nsl],
                            start=(t == 0), stop=(t == OT - 1))
                    # alternate engines so consecutive PSUM drains pipeline
                    if ot % 2 == 0:
                        nc.vector.tensor_copy(fin[:, ot, :], ps[:])
                    else:
                        nc.scalar.copy(fin[:, ot, :], ps[:])
                    step = 2 if tail else half
                    if (ot + 1) % step == 0:
                        # batched writeback: one DMA issue beats per-chunk
                        # 700ns-spaced issues; the tail uses quarters so the
                        # last transfer trails the last drain minimally
                        lo = ot + 1 - step
                        nc.sync.dma_start(
                            yt[lo * P:(ot + 1) * P, nsl].rearrange(
                                "(a p) s -> p a s", a=step),
                            fin[:, lo:ot + 1, :])

            for j in range(NT):
                # prefetch chunk j+1 while computing chunk j
                if j + 1 < NT:
                    jsl = slice((j + 1) * NQ, (j + 2) * NQ)
                    nc.sync.dma_start(
                        xt_sb[:, :, (j + 1) % 2, :],
                        xt[:, jsl].rearrange("(k p) s -> p k s", k=KT))
                    nc.sync.dma_start(cos_sb[:, jsl], cosr[:, jsl])
                    nc.sync.dma_start(sin_sb[:, jsl], sinr[:, jsl])
                if j == 0:
                    nc.sync.dma_start(wo_sb[:, :, :],
                                      wo[:, :, :].rearrange("t p d -> p t d"))
                # pair 0's q/k first: its rope chain gates the first scores;
                # pair 1's projection is deferred past attn0 so its rope
                # chain (DVE/Pool) doesn't queue ahead of attn0's masks
                _proj_qk(0, j)
                for st in range(4 * j, 4 * j + 4):
                    _proj_v(st)
                if j > 0:
                    _norm_b(1, j - 1, *saved)   # recips long done: no stall
                st0 = _attn(0, j)
                _proj_qk(1, j)
                if j > 0:
                    _wo_col(j - 1)   # fills PE while pair 0's recips run
                _norm_b(0, j, *st0)
                saved = _attn(1, j)
                if j == NT - 1:
                    tail_rcs = _recips(saved[0])
            _norm_b(1, NT - 1, *saved, tail=True, rcs=tail_rcs)
            _wo_col(NT - 1, tail=True)

    nc.compile()
    return nc


_NC_CACHE = {}


def _get_nc():
    if "nc" not in _NC_CACHE:
        _NC_CACHE["nc"] = build_nc()
    return _NC_CACHE["nc"]


_HALF = DK // 2
# 16-deep interleave: quadrant q holds freqs 16q..16q+15, x1 rows then x2 rows
_PERM = np.array([2 * (16 * q + i) + c
                  for q in (0, 1) for c in (0, 1) for i in range(16)])


BF16NP = ml_dtypes.bfloat16


def _prep_qk(W, heads):
    """Per-head RoPE-permuted projection weights (transposed for lhsT)."""
    Wh = W.reshape(H, DK, D)[heads][:, _PERM, :]                    # [NH, DK, D]
    return np.ascontiguousarray(Wh.reshape(NH * DK, D).T.astype(BF16NP))


def make_in_maps(x, token_positions, Wq, Wk, Wv, Wo):
    x = np.asarray(x, dtype=np.float32)
    Wq = np.asarray(Wq, dtype=np.float32)
    Wk = np.asarray(Wk, dtype=np.float32)
    Wv = np.asarray(Wv, dtype=np.float32)
    Wo = np.asarray(Wo, dtype=np.float32)
    pos = np.asarray(token_positions)

    j = np.arange(_HALF, dtype=np.float64)
    inv_freq = ROPE_THETA ** (-2.0 * j / DK)                        # [32]

    in_maps = []
    for core in range(NCORES):
        b = core // GROUPS
        g = core % GROUPS
        heads = list(range(g * NH, (g + 1) * NH))
        wq_ = _prep_qk(Wq, heads)
        wk_ = _prep_qk(Wk, heads)
        wv_ = np.ascontiguousarray(Wv.reshape(H, DK, D)[heads].reshape(NH * DK, D).T.astype(BF16NP))
        wo_ = np.ascontiguousarray(Wo.T.reshape(H, DK, D)[heads].reshape(OT, P, D).astype(BF16NP))
        ang = np.outer(inv_freq, pos[b].astype(np.float64))          # [32, S]
        cos32 = np.cos(ang)
        sin32 = np.sin(ang)
        # row layout per 64-block: [f0..15, f0..15, f16..31, f16..31]
        cos64 = np.concatenate([cos32[0:16], cos32[0:16],
                                cos32[16:32], cos32[16:32]])
        sin64 = np.concatenate([-sin32[0:16], sin32[0:16],
                                -sin32[16:32], sin32[16:32]])
        cosr = np.tile(cos64, (2, 1)).astype(np.float32)             # [128, S]
        sinr = np.tile(sin64, (2, 1)).astype(np.float32)
        in_maps.append({
            "xt": np.ascontiguousarray(x[b].T.astype(BF16NP)),
            "wq": wq_, "wk": wk_, "wv": wv_, "wo": wo_,
            "cosr": cosr, "sinr": sinr,
            "ones": np.ones((P, DK), dtype=np.float32),
        })
    return in_maps


def _gather(results):
    outs = [np.asarray(r["yt"], dtype=np.float32) for r in results]
    y = np.stack([
        sum(outs[b * GROUPS + 1: (b + 1) * GROUPS], outs[b * GROUPS]).T
        for b in range(B)
    ])
    return np.ascontiguousarray(y)


def kernel(x, token_positions, Wq, Wk, Wv, Wo):
    in_maps = make_in_maps(x, token_positions, Wq, Wk, Wv, Wo)
    res = run_bass_kernel_spmd(_get_nc(), in_maps, core_ids=list(range(NCORES)))
    return _gather(res.results)


def kernel_traced(x, token_positions, Wq, Wk, Wv, Wo, **kwargs):
    """Like kernel() but with NTFF tracing; returns (output, BassKernelResults)."""
    in_maps = make_in_maps(x, token_positions, Wq, Wk, Wv, Wo)
    res = run_bass_kernel_spmd(_get_nc(), in_maps, core_ids=list(range(NCORES)),
                               trace=True, **kwargs)
    return _gather(res.results), res


# revision 75
# speedup vs baseline: 1.0366x; 1.0018x over previous
"""Multi-head self-attention (RoPE, causal softmax) — Trainium2 Bass kernel.

Sharding over 8 NeuronCores: batch (2) x head-groups (16 heads / 4 groups).
Each core handles one batch element and 4 heads. Single software-pipelined
phase, streamed per query/key column block j (512 positions):

  - x / weights / cos/sin stream in as few batched multi-tile DMAs (the
    565ns-per-DMA descriptor-issue rate, not bytes, paces the prologue);
    x and the weights travel in bf16, halving prologue bytes; the first
    projection matmul starts ~3us in instead of ~36us
  - q/k projections for chunk j (head pairs stacked on partitions), RoPE
    via DVE stream_shuffle: head dims are host-permuted into a 16-deep
    interleave (x1/x2 component blocks of 16 rows inside each 32-partition
    quadrant) so the rotation partner is a within-quadrant 16-row swap —
    no second "rotated weights" matmul stream on the PE
  - v projection for key blocks 4j..4j+3 into v_aug (extra ones-column so
    the softmax denominator falls out of the attn@v matmul, PSUM row 64)
  - causal attention for column j, both head pairs: scoresT[m,n] blocks
    on PE writing a paired PSUM tile [128, 2, w], exp on ACT (scale=1/8
    fused) over both heads at once, causal mask via one gpsimd
    affine_select (pattern [[0,2],[1,w]]), attn@v per head; diagonal
    (masked) blocks are computed first so the end-of-loop drain consumes
    mask-free full blocks; scores run LAG blocks ahead of their attn@v
    consumers so the in-order PE never stalls on the exp->mask chain
  - per-head normalize, split in two stages: (A) DVE reciprocals of the
    denominator rows right after the i-loop, (B) ones-column PE broadcast
    matmul + drain + multiplies, deferred until fresh matmul work (next
    chunk's projections, wo) covers the latency; the odd head's rows reach
    partitions 64-127 via a gpsimd cross-partition copy (gpsimd may not
    touch PSUM, DVE may not cross partitions; this split satisfies both)
  - output projection wo(j-1) is emitted between attn pair 0 and pair 1 of
    chunk j; the final wo borrows the idle score-pool PSUM to run two
    output chunks in flight; partials are written to DRAM in bf16 via two
    batched half-column DMAs per chunk (host accumulates in fp32)

Attention matmuls run in float32r (full rate on the PE array); the x/W
projections run in bf16. Measured rel err ~3.7e-3 vs the fp32 reference.
"""

from contextlib import ExitStack

import ml_dtypes
import numpy as np

import concourse.bass as bass
import concourse.bacc as bacc
import concourse.tile as tile
from concourse import mybir
from concourse.bass_utils import run_bass_kernel_spmd

# problem shape (hardcoded: graded standalone)
B, S, D, H, DK = 2, 2048, 1024, 16, 64
NCORES = 8
GROUPS = NCORES // B  # 4 head-groups (cores) per batch element
NH = H // GROUPS      # 4 heads per core
ROPE_THETA = 10000.0

P = 128
NQ = 512              # query-block (matmul moving free dim)
NT = S // NQ          # 4 query blocks
NM = S // P           # 16 key blocks
KT = D // P           # 8 contraction tiles for the x-projections
OT = NH * DK // P     # 2 stacked head-pair tiles for q/k
LAG = 7               # score blocks emitted ahead of attn@v consumers

F32 = mybir.dt.float32
F32R = mybir.dt.float32r
BF16 = mybir.dt.bfloat16

# swap the two 16-row component blocks inside each 32-partition quadrant
SHUF_MASK = list(range(16, 32)) + list(range(0, 16))


def build_nc():
    nc = bacc.Bacc("TRN2", target_bir_lowering=False, debug=False)

    xt = nc.dram_tensor("xt", [D, S], BF16, kind="ExternalInput")        # x[b].T
    wq = nc.dram_tensor("wq", [D, NH * DK], BF16, kind="ExternalInput")  # perm'd, T
    wk = nc.dram_tensor("wk", [D, NH * DK], BF16, kind="ExternalInput")
    wv = nc.dram_tensor("wv", [D, NH * DK], BF16, kind="ExternalInput")
    wo = nc.dram_tensor("wo", [OT, P, D], BF16, kind="ExternalInput")   # pair-stacked
    cosr = nc.dram_tensor("cosr", [P, S], F32, kind="ExternalInput")
    sinr = nc.dram_tensor("sinr", [P, S], F32, kind="ExternalInput")    # sign-folded
    yt = nc.dram_tensor("yt", [D, S], BF16, kind="ExternalOutput")      # partial y.T
    ones_dram = nc.dram_tensor("ones", [P, DK], F32R, kind="ExternalInput")

    with tile.TileContext(nc) as tc, ExitStack() as ctx, \
            nc.allow_low_precision(reason="float32r matmul inputs and bf16 partial outputs are within tolerance"):
        # ---- persistent SBUF (whole kernel) ----
        persist = ctx.enter_context(tc.tile_pool(name="persist", bufs=1))
        cos_sb = persist.tile([P, S], F32, name="cos_sb")
        sin_sb = persist.tile([P, S], F32, name="sin_sb")
        ones_sb = persist.tile([P, DK], F32R, name="ones_sb")
        qtr_sb = persist.tile([P, OT, S], F32R, name="qtr_sb")    # rope(q)^T pairs
        ktr_sb = persist.tile([P, OT, S], F32R, name="ktr_sb")
        vaug_sb = persist.tile([P, NM, NH, DK + 1], F32R, name="vaug_sb")
        outt_sb = persist.tile([P, OT, S], BF16, name="outt_sb")  # out^T pairs
        wq_sb = persist.tile([P, KT, NH * DK], BF16, name="wq_sb")
        wk_sb = persist.tile([P, KT, NH * DK], BF16, name="wk_sb")
        wv_sb = persist.tile([P, KT, NH * DK], BF16, name="wv_sb")
        wo_sb = persist.tile([P, OT, D], BF16, name="wo_sb")
        xt_sb = persist.tile([P, KT, 2, NQ], BF16, name="xt_sb")  # j-chunk double buffer

        # prologue DMAs: batched multi-tile transfers — the SP queue's
        # 565ns/DMA issue rate, not bytes, paces the prologue, so one
        # 3-dim-AP DMA per tensor beats eight per-tile DMAs. xt rides the
        # scalar queue so both descriptor generators run in parallel.
        nc.sync.dma_start(wq_sb[:, 0:2, :],
                          wq[0:2 * P, :].rearrange("(k p) c -> p k c", k=2))
        nc.sync.dma_start(wq_sb[:, 2:KT // 2, :],
                          wq[2 * P:D // 2, :].rearrange("(k p) c -> p k c", k=2))
        nc.sync.dma_start(wq_sb[:, KT // 2:, :],
                          wq[D // 2:, :].rearrange("(k p) c -> p k c", k=KT // 2))
        nc.sync.dma_start(wk_sb[:, 0:KT // 2, :],
                          wk[0:D // 2, :].rearrange("(k p) c -> p k c", k=KT // 2))
        nc.sync.dma_start(wk_sb[:, KT // 2:, :],
                          wk[D // 2:, :].rearrange("(k p) c -> p k c", k=KT // 2))
        nc.scalar.dma_start(xt_sb[:, 0:2, 0, :],
                            xt[0:2 * P, 0:NQ].rearrange("(k p) s -> p k s", k=2))
        nc.scalar.dma_start(xt_sb[:, 2:KT // 2, 0, :],
                            xt[2 * P:D // 2, 0:NQ].rearrange("(k p) s -> p k s", k=2))
        nc.scalar.dma_start(xt_sb[:, KT // 2:, 0, :],
                            xt[D // 2:, 0:NQ].rearrange("(k p) s -> p k s", k=KT // 2))
        nc.sync.dma_start(cos_sb[:, 0:NQ], cosr[:, 0:NQ])
        nc.sync.dma_start(sin_sb[:, 0:NQ], sinr[:, 0:NQ])
        nc.sync.dma_start(wv_sb[:, :, :], wv[:, :].rearrange("(k p) c -> p k c", k=KT))
        # ones for the denominator broadcast; the scattered v_aug ones
        # column is filled by a cheap DVE copy (a strided DMA costs ~3.6us)
        nc.sync.dma_start(ones_sb[:], ones_dram[:, :])
        nc.vector.tensor_copy(vaug_sb[:, :, :, DK],
                              ones_sb[:].rearrange("p (a b) -> p a b", a=NM))

        pools = (
            tc.tile_pool(name="pp_ps", bufs=2, space="PSUM"),      # proj/v/bcast/wo
            tc.tile_pool(name="score_ps", bufs=2, space="PSUM"),   # paired scores
            tc.tile_pool(name="oaug_ps", bufs=2, space="PSUM"),
            tc.tile_pool(name="rope_sb", bufs=9),
            tc.tile_pool(name="exp_sb", bufs=10),
            tc.tile_pool(name="norm_sb", bufs=12),
            tc.tile_pool(name="fin_sb", bufs=2),
        )
        with pools[0] as pp_ps, pools[1] as score_ps, pools[2] as oaug_ps, \
                pools[3] as rope_sb, pools[4] as exp_sb, pools[5] as norm_sb, \
                pools[6] as fin_sb:

            def _proj_qk(t, j):
                """q+k projection + rope for head pair t, chunk j.

                Pair 0 (emitted while the score pool is idle) interleaves the
                q/k k-loops in one paired score-pool tile — matching the
                prologue DMA triplet pacing. Pair 1 is emitted during attn0's
                score cycling, so it uses sequential k-loops on single-bank
                pp tiles instead of stealing a score buffer."""
                csl = slice(j * NQ, (j + 1) * NQ)
                if t == 0:
                    # sequential q-then-k loops in one paired score-pool tile
                    # (idle during projections): the q loop depends only on
                    # wq+xt, so the prologue feed keeps it PE-bound while wk
                    # streams in behind it
                    ps2 = score_ps.tile([P, 2, NQ], F32, name="proj_ps", tag="sc")
                    pss = (ps2[:, 0, :], ps2[:, 1, :])
                    for qk, w_sb in ((0, wq_sb), (1, wk_sb)):
                        for k in range(KT):
                            nc.tensor.matmul(
                                pss[qk],
                                lhsT=w_sb[:, k, t * P:(t + 1) * P],
                                rhs=xt_sb[:, k, j % 2, :],
                                start=(k == 0), stop=(k == KT - 1))
                else:
                    psq = pp_ps.tile([P, NQ], F32, name="projq_ps", tag="pp")
                    psk = pp_ps.tile([P, NQ], F32, name="projk_ps", tag="pp")
                    pss = (psq[:], psk[:])
                    for qk, w_sb in ((0, wq_sb), (1, wk_sb)):
                        for k in range(KT):
                            nc.tensor.matmul(
                                pss[qk],
                                lhsT=w_sb[:, k, t * P:(t + 1) * P],
                                rhs=xt_sb[:, k, j % 2, :],
                                start=(k == 0), stop=(k == KT - 1))
                for qk, dst in ((0, qtr_sb), (1, ktr_sb)):
                    ps = pss[qk]
                    shuf = rope_sb.tile([P, NQ], F32, name="shuf", tag="rope")
                    t1 = rope_sb.tile([P, NQ], F32, name="rope_a", tag="rope")
                    t2 = rope_sb.tile([P, NQ], F32, name="rope_b", tag="rope")
                    nc.vector.stream_shuffle(shuf[:], ps, SHUF_MASK)
                    nc.vector.tensor_mul(t1[:], ps, cos_sb[:, csl])
                    nc.gpsimd.tensor_mul(t2[:], shuf[:], sin_sb[:, csl])
                    nc.vector.tensor_add(dst[:, t, csl], t1[:], t2[:])

            def _proj_v(st, on_dve=False):
                psv = pp_ps.tile([P, NH * DK], F32, name="projv_ps", tag="pp")
                for k in range(KT):
                    nc.tensor.matmul(
                        psv[:],
                        lhsT=xt_sb[:, k, (st // 4) % 2, (st % 4) * P:(st % 4 + 1) * P],
                        rhs=wv_sb[:, k, :],
                        start=(k == 0), stop=(k == KT - 1))
                dst = vaug_sb[:, st, :, 0:DK]
                src = psv[:].rearrange("p (h c) -> p h c", h=NH)
                if on_dve:
                    nc.vector.tensor_copy(dst, src)
                else:
                    nc.scalar.copy(dst, src)

            def _attn(pr, j):
                """Causal attention for head pair pr, query chunk j."""
                imax = 4 * j + 3
                nsl = slice(j * NQ, (j + 1) * NQ)
                oa = [oaug_ps.tile([DK + 1, NQ], F32, name="oaug") for _ in range(2)]

                def _scores(i, start, stop):
                    # diagonal blocks (offset d = i - 4j): columns below
                    # 128*d are fully causal-masked — compute only [c0:512).
                    # Exception: d=3 would give N=128, which fp32r runs at a
                    # 4x penalty (ap_size < 256); widening it to N=256 halves
                    # its real cost — the extra columns are fully masked and
                    # contribute exact zeros to the attn@v accumulation
                    d = max(i - 4 * j, 0)
                    c0 = 2 * P if d == 3 else P * d
                    w = NQ - c0
                    sc = score_ps.tile([P, 2, NQ], F32, name="score", tag="sc")
                    for hh in range(2):
                        hb = hh * DK
                        nc.tensor.matmul(
                            sc[:, hh, 0:w],
                            lhsT=ktr_sb[hb:hb + DK, pr, i * P:(i + 1) * P],
                            rhs=qtr_sb[hb:hb + DK, pr,
                                       j * NQ + c0:(j + 1) * NQ],
                            start=True, stop=True)
                    eb = exp_sb.tile([P, 2, NQ], F32R, name="expblk")
                    nc.scalar.activation(
                        eb[:, :, 0:w], sc[:, :, 0:w],
                        mybir.ActivationFunctionType.Exp,
                        scale=float(1.0 / np.sqrt(DK)))
                    if i >= 4 * j:   # diagonal block: causal mask, both heads
                        # masked cells satisfy col - base < p <= 127: only
                        # the first 128+(-base) columns can ever be zeroed —
                        # restricting the AP there cuts the Pool op (and the
                        # chain the attn@v waits on); later columns are
                        # already correct in SBUF
                        base = -P if d == 3 else 0
                        mw = min(w, P - base)
                        nc.gpsimd.affine_select(
                            out=eb[:, :, 0:mw], in_=eb[:, :, 0:mw],
                            compare_op=mybir.AluOpType.is_ge,
                            fill=0.0,
                            base=base,
                            channel_multiplier=-1,
                            pattern=[[0, 2], [1, mw]])
                    return (eb, c0, w, start, stop)

                def _attnv(i, blk):
                    eb, c0, w, start, stop = blk
                    for hh in range(2):
                        nc.tensor.matmul(
                            oa[hh][:, c0:NQ],
                            lhsT=vaug_sb[:, i, 2 * pr + hh, :],
                            rhs=eb[:, hh, 0:w],
                            start=start, stop=stop)

                # diagonal blocks first: their exp->mask chains are the
                # longest, so the drain attn@v's at the end consume mask-free
                # full blocks instead of waiting on fresh masks
                i_order = list(range(4 * j, imax + 1)) + list(range(0, 4 * j))
                pending = {}
                for pos, i in enumerate(i_order):
                    pending[i] = _scores(i, start=(pos == 0),
                                         stop=(pos == imax))
                    if pos >= LAG:
                        ii = i_order[pos - LAG]
                        _attnv(ii, pending.pop(ii))
                for ii in i_order[max(0, imax + 1 - LAG):]:
                    _attnv(ii, pending.pop(ii))

                return (oa,)

            def _recips(oa):
                rcs = []
                for hh in (1, 0):
                    rc = norm_sb.tile([DK + 1, NQ], F32R, name="recip", tag="nrm")
                    nc.vector.reciprocal(rc[DK:DK + 1, :], oa[hh][DK:DK + 1, :])
                    rcs.append((hh, rc))
                return rcs

            def _norm_b(pr, j, oa, tail=False, rcs=None):
                """Normalize: reciprocal of the denominator row (row 64, the
                ones column of v), broadcast, multiply into outt. Emitted
                behind filler matmul work so the PE never waits the chain."""
                nsl = slice(j * NQ, (j + 1) * NQ)
                if rcs is None:
                    rcs = _recips(oa)
                bss = []
                for hh, rc in rcs:
                    bc = pp_ps.tile([DK, NQ], F32, name="bcast_ps", tag="pp")
                    nc.tensor.matmul(
                        bc[:],
                        lhsT=ones_sb[DK:DK + 1, :],
                        rhs=rc[DK:DK + 1, :],
                        start=True, stop=True)
                    bs = norm_sb.tile([DK, NQ], F32, name="bcast_sb", tag="nrm")
                    if tail:
                        # final chain: ACT is idle, shorten the DVE span
                        nc.scalar.copy(bs[:], bc[:])
                    else:
                        nc.vector.tensor_copy(bs[:], bc[:])
                    bss.append((hh, bs))
                for hh, bs in bss:
                    if hh == 0:
                        nc.vector.tensor_mul(
                            outt_sb[0:DK, pr, nsl], oa[hh][0:DK, :], bs[:])
                    else:
                        # odd head: normalize at base 0 (DVE cannot cross
                        # partitions), then gpsimd places rows 64-127
                        ot_tmp = norm_sb.tile([DK, NQ], BF16, name="ot_tmp",
                                              tag="nrm")
                        nc.vector.tensor_mul(ot_tmp[:], oa[hh][0:DK, :], bs[:])
                        nc.gpsimd.tensor_copy(outt_sb[DK:P, pr, nsl], ot_tmp[:])

            def _wo_col(j, tail=False):
                nsl = slice(j * NQ, (j + 1) * NQ)
                half = D // P // 2
                fin = fin_sb.tile([P, D // P, NQ], BF16, name="fin")
                for ot in range(D // P):
                    if tail and ot % 2 == 1:
                        # attention is over: borrow the idle score pool so
                        # two output chunks are in flight
                        ps2 = score_ps.tile([P, 2, NQ], F32, name="wo_ps2",
                                            tag="sc")
                        ps = ps2[:, 0, :]
                    else:
                        ps = pp_ps.tile([P, NQ], F32, name="wo_ps", tag="pp")
                    for t in range(OT):
                        nc.tensor.matmul(
                            ps[:],
                            lhsT=wo_sb[:, t, ot * P:(ot + 1) * P],
                            rhs=outt_sb[:, t, nsl],
                            start=(t == 0), stop=(t == OT - 1))
                    # alternate engines so consecutive PSUM drains pipeline
                    if ot % 2 == 0:
                        nc.vector.tensor_copy(fin[:, ot, :], ps[:])
                    else:
                        nc.scalar.copy(fin[:, ot, :], ps[:])
                    step = 2 if tail else half
                    if (ot + 1) % step == 0:
                        # batched writeback: one DMA issue beats per-chunk
                        # 700ns-spaced issues; the tail uses quarters so the
                        # last transfer trails the last drain minimally
                        lo = ot + 1 - step
                        nc.sync.dma_start(
                            yt[lo * P:(ot + 1) * P, nsl].rearrange(
                                "(a p) s -> p a s", a=step),
                            fin[:, lo:ot + 1, :])

            for j in range(NT):
                # prefetch chunk j+1 while computing chunk j
                if j + 1 < NT:
                    jsl = slice((j + 1) * NQ, (j + 2) * NQ)
                    nc.sync.dma_start(
                        xt_sb[:, :, (j + 1) % 2, :],
                        xt[:, jsl].rearrange("(k p) s -> p k s", k=KT))
                    nc.sync.dma_start(cos_sb[:, jsl], cosr[:, jsl])
                    nc.sync.dma_start(sin_sb[:, jsl], sinr[:, jsl])
                if j == 0:
                    nc.sync.dma_start(wo_sb[:, :, :],
                                      wo[:, :, :].rearrange("t p d -> p t d"))
                # pair 0's q/k first: its rope chain gates the first scores;
                # pair 1's projection is deferred past attn0 so its rope
                # chain (DVE/Pool) doesn't queue ahead of attn0's masks
                _proj_qk(0, j)
                for st in range(4 * j, 4 * j + 4):
                    _proj_v(st)
                if j > 0:
                    _norm_b(1, j - 1, *saved)   # recips long done: no stall
                st0 = _attn(0, j)
                _proj_qk(1, j)
                if j > 0:
                    _wo_col(j - 1)   # fills PE while pair 0's recips run
                _norm_b(0, j, *st0)
                saved = _attn(1, j)
                if j == NT - 1:
                    tail_rcs = _recips(saved[0])
            _norm_b(1, NT - 1, *saved, tail=True, rcs=tail_rcs)
            _wo_col(NT - 1, tail=True)

    nc.compile()
    return nc


_NC_CACHE = {}


def _get_nc():
    if "nc" not in _NC_CACHE:
        _NC_CACHE["nc"] = build_nc()
    return _NC_CACHE["nc"]


_HALF = DK // 2
# 16-deep interleave: quadrant q holds freqs 16q..16q+15, x1 rows then x2 rows
_PERM = np.array([2 * (16 * q + i) + c
                  for q in (0, 1) for c in (0, 1) for i in range(16)])


BF16NP = ml_dtypes.bfloat16


def _prep_qk(W, heads):
    """Per-head RoPE-permuted projection weights (transposed for lhsT)."""
    Wh = W.reshape(H, DK, D)[heads][:, _PERM, :]                    # [NH, DK, D]
    return np.ascontiguousarray(Wh.reshape(NH * DK, D).T.astype(BF16NP))


def make_in_maps(x, token_positions, Wq, Wk, Wv, Wo):
    x = np.asarray(x, dtype=np.float32)
    Wq = np.asarray(Wq, dtype=np.float32)
    Wk = np.asarray(Wk, dtype=np.float32)
    Wv = np.asarray(Wv, dtype=np.float32)
    Wo = np.asarray(Wo, dtype=np.float32)
    pos = np.asarray(token_positions)

    j = np.arange(_HALF, dtype=np.float64)
    inv_freq = ROPE_THETA ** (-2.0 * j / DK)                        # [32]

    in_maps = []
    for core in range(NCORES):
        b = core // GROUPS
        g = core % GROUPS
        heads = list(range(g * NH, (g + 1) * NH))
        wq_ = _prep_qk(Wq, heads)
        wk_ = _prep_qk(Wk, heads)
        wv_ = np.ascontiguousarray(Wv.reshape(H, DK, D)[heads].reshape(NH * DK, D).T.astype(BF16NP))
        wo_ = np.ascontiguousarray(Wo.T.reshape(H, DK, D)[heads].reshape(OT, P, D).astype(BF16NP))
        ang = np.outer(inv_freq, pos[b].astype(np.float64))          # [32, S]
        cos32 = np.cos(ang)
        sin32 = np.sin(ang)
        # row layout per 64-block: [f0..15, f0..15, f16..31, f16..31]
        cos64 = np.concatenate([cos32[0:16], cos32[0:16],
                                cos32[16:32], cos32[16:32]])
        sin64 = np.concatenate([-sin32[0:16], sin32[0:16],
                                -sin32[16:32], sin32[16:32]])
        cosr = np.tile(cos64, (2, 1)).astype(np.float32)             # [128, S]
        sinr = np.tile(sin64, (2, 1)).astype(np.float32)
        in_maps.append({
            "xt": np.ascontiguousarray(x[b].T.astype(BF16NP)),
            "wq": wq_, "wk": wk_, "wv": wv_, "wo": wo_,
            "cosr": cosr, "sinr": sinr,
            "ones": np.ones((P, DK), dtype=np.float32),
        })
    return in_maps


def _gather(results):
    outs = [np.asarray(r["yt"], dtype=np.float32) for r in results]
    y = np.stack([
        sum(outs[b * GROUPS + 1: (b + 1) * GROUPS], outs[b * GROUPS]).T
        for b in range(B)
    ])
    return np.ascontiguousarray(y)


def kernel(x, token_positions, Wq, Wk, Wv, Wo):
    in_maps = make_in_maps(x, token_positions, Wq, Wk, Wv, Wo)
    res = run_bass_kernel_spmd(_get_nc(), in_maps, core_ids=list(range(NCORES)))
    return _gather(res.results)


def kernel_traced(x, token_positions, Wq, Wk, Wv, Wo, **kwargs):
    """Like kernel() but with NTFF tracing; returns (output, BassKernelResults)."""
    in_maps = make_in_maps(x, token_positions, Wq, Wk, Wv, Wo)
    res = run_bass_kernel_spmd(_get_nc(), in_maps, core_ids=list(range(NCORES)),
                               trace=True, **kwargs)
    return _gather(res.results), res


# revision 78
# speedup vs baseline: 1.0464x; 1.0095x over previous
"""Multi-head self-attention (RoPE, causal softmax) — Trainium2 Bass kernel.

Sharding over 8 NeuronCores: batch (2) x head-groups (16 heads / 4 groups).
Each core handles one batch element and 4 heads. Single software-pipelined
phase, streamed per query/key column block j (512 positions):

  - x / weights / cos/sin stream in as few batched multi-tile DMAs (the
    565ns-per-DMA descriptor-issue rate, not bytes, paces the prologue);
    x and the weights travel in bf16, halving prologue bytes; the first
    projection matmul starts ~3us in instead of ~36us
  - q/k projections for chunk j (head pairs stacked on partitions), RoPE
    via DVE stream_shuffle: head dims are host-permuted into a 16-deep
    interleave (x1/x2 component blocks of 16 rows inside each 32-partition
    quadrant) so the rotation partner is a within-quadrant 16-row swap —
    no second "rotated weights" matmul stream on the PE
  - v projection for key blocks 4j..4j+3 into v_aug (extra ones-column so
    the softmax denominator falls out of the attn@v matmul, PSUM row 64)
  - causal attention for column j, both head pairs: scoresT[m,n] blocks
    on PE writing a paired PSUM tile [128, 2, w], exp on ACT (scale=1/8
    fused) over both heads at once, causal mask via one gpsimd
    affine_select (pattern [[0,2],[1,w]]), attn@v per head; diagonal
    (masked) blocks are computed first so the end-of-loop drain consumes
    mask-free full blocks; scores run LAG blocks ahead of their attn@v
    consumers so the in-order PE never stalls on the exp->mask chain
  - per-head normalize, split in two stages: (A) DVE reciprocals of the
    denominator rows right after the i-loop, (B) ones-column PE broadcast
    matmul + drain + multiplies, deferred until fresh matmul work (next
    chunk's projections, wo) covers the latency; the odd head's rows reach
    partitions 64-127 via a gpsimd cross-partition copy (gpsimd may not
    touch PSUM, DVE may not cross partitions; this split satisfies both)
  - output projection wo(j-1) is emitted between attn pair 0 and pair 1 of
    chunk j; the final wo borrows the idle score-pool PSUM to run two
    output chunks in flight; partials are written to DRAM in bf16 via two
    batched half-column DMAs per chunk (host accumulates in fp32)

Attention matmuls run in float32r (full rate on the PE array); the x/W
projections run in bf16. Measured rel err ~3.7e-3 vs the fp32 reference.
"""

from contextlib import ExitStack

import ml_dtypes
import numpy as np

import concourse.bass as bass
import concourse.bacc as bacc
import concourse.tile as tile
from concourse import mybir
from concourse.bass_utils import run_bass_kernel_spmd

# problem shape (hardcoded: graded standalone)
B, S, D, H, DK = 2, 2048, 1024, 16, 64
NCORES = 8
GROUPS = NCORES // B  # 4 head-groups (cores) per batch element
NH = H // GROUPS      # 4 heads per core
ROPE_THETA = 10000.0

P = 128
NQ = 512              # query-block (matmul moving free dim)
NT = S // NQ          # 4 query blocks
NM = S // P           # 16 key blocks
KT = D // P           # 8 contraction tiles for the x-projections
OT = NH * DK // P     # 2 stacked head-pair tiles for q/k
LAG = 7               # score blocks emitted ahead of attn@v consumers

F32 = mybir.dt.float32
F32R = mybir.dt.float32r
BF16 = mybir.dt.bfloat16

# swap the two 16-row component blocks inside each 32-partition quadrant
SHUF_MASK = list(range(16, 32)) + list(range(0, 16))


def build_nc():
    nc = bacc.Bacc("TRN2", target_bir_lowering=False, debug=False)

    xt = nc.dram_tensor("xt", [D, S], BF16, kind="ExternalInput")        # x[b].T
    wq = nc.dram_tensor("wq", [D, NH * DK], BF16, kind="ExternalInput")  # perm'd, T
    wk = nc.dram_tensor("wk", [D, NH * DK], BF16, kind="ExternalInput")
    wv = nc.dram_tensor("wv", [D, NH * DK], BF16, kind="ExternalInput")
    wo = nc.dram_tensor("wo", [OT, P, D], BF16, kind="ExternalInput")   # pair-stacked
    cosr = nc.dram_tensor("cosr", [P, S], F32, kind="ExternalInput")
    sinr = nc.dram_tensor("sinr", [P, S], F32, kind="ExternalInput")    # sign-folded
    yt = nc.dram_tensor("yt", [D, S], BF16, kind="ExternalOutput")      # partial y.T
    ones_dram = nc.dram_tensor("ones", [P, DK], F32R, kind="ExternalInput")

    with tile.TileContext(nc) as tc, ExitStack() as ctx, \
            nc.allow_low_precision(reason="float32r matmul inputs and bf16 partial outputs are within tolerance"):
        # ---- persistent SBUF (whole kernel) ----
        persist = ctx.enter_context(tc.tile_pool(name="persist", bufs=1))
        cos_sb = persist.tile([P, S], F32, name="cos_sb")
        sin_sb = persist.tile([P, S], F32, name="sin_sb")
        ones_sb = persist.tile([P, DK], F32R, name="ones_sb")
        qtr_sb = persist.tile([P, OT, S], F32R, name="qtr_sb")    # rope(q)^T pairs
        ktr_sb = persist.tile([P, OT, S], F32R, name="ktr_sb")
        vaug_sb = persist.tile([P, NM, NH, DK + 1], F32R, name="vaug_sb")
        outt_sb = persist.tile([P, OT, S], BF16, name="outt_sb")  # out^T pairs
        wq_sb = persist.tile([P, KT, NH * DK], BF16, name="wq_sb")
        wk_sb = persist.tile([P, KT, NH * DK], BF16, name="wk_sb")
        wv_sb = persist.tile([P, KT, NH * DK], BF16, name="wv_sb")
        wo_sb = persist.tile([P, OT, D], BF16, name="wo_sb")
        xt_sb = persist.tile([P, KT, 2, NQ], BF16, name="xt_sb")  # j-chunk double buffer

        # prologue DMAs: batched multi-tile transfers — the SP queue's
        # 565ns/DMA issue rate, not bytes, paces the prologue, so one
        # 3-dim-AP DMA per tensor beats eight per-tile DMAs. xt rides the
        # scalar queue so both descriptor generators run in parallel.
        nc.sync.dma_start(wq_sb[:, 0:2, :],
                          wq[0:2 * P, :].rearrange("(k p) c -> p k c", k=2))
        nc.sync.dma_start(wq_sb[:, 2:KT // 2, :],
                          wq[2 * P:D // 2, :].rearrange("(k p) c -> p k c", k=2))
        nc.sync.dma_start(wq_sb[:, KT // 2:, :],
                          wq[D // 2:, :].rearrange("(k p) c -> p k c", k=KT // 2))
        nc.sync.dma_start(wk_sb[:, 0:KT // 2, :],
                          wk[0:D // 2, :].rearrange("(k p) c -> p k c", k=KT // 2))
        nc.sync.dma_start(wk_sb[:, KT // 2:, :],
                          wk[D // 2:, :].rearrange("(k p) c -> p k c", k=KT // 2))
        nc.scalar.dma_start(xt_sb[:, 0:2, 0, :],
                            xt[0:2 * P, 0:NQ].rearrange("(k p) s -> p k s", k=2))
        nc.scalar.dma_start(xt_sb[:, 2:KT // 2, 0, :],
                            xt[2 * P:D // 2, 0:NQ].rearrange("(k p) s -> p k s", k=2))
        nc.scalar.dma_start(xt_sb[:, KT // 2:, 0, :],
                            xt[D // 2:, 0:NQ].rearrange("(k p) s -> p k s", k=KT // 2))
        nc.sync.dma_start(cos_sb[:, 0:NQ], cosr[:, 0:NQ])
        nc.sync.dma_start(sin_sb[:, 0:NQ], sinr[:, 0:NQ])
        nc.sync.dma_start(wv_sb[:, :, :], wv[:, :].rearrange("(k p) c -> p k c", k=KT))
        # ones for the denominator broadcast; the scattered v_aug ones
        # column is filled by a cheap DVE copy (a strided DMA costs ~3.6us)
        nc.sync.dma_start(ones_sb[:], ones_dram[:, :])
        nc.vector.tensor_copy(vaug_sb[:, :, :, DK],
                              ones_sb[:].rearrange("p (a b) -> p a b", a=NM))

        pools = (
            tc.tile_pool(name="pp_ps", bufs=2, space="PSUM"),      # proj/v/bcast/wo
            tc.tile_pool(name="score_ps", bufs=2, space="PSUM"),   # paired scores
            tc.tile_pool(name="oaug_ps", bufs=2, space="PSUM"),
            tc.tile_pool(name="rope_sb", bufs=9),
            tc.tile_pool(name="exp_sb", bufs=10),
            tc.tile_pool(name="norm_sb", bufs=12),
            tc.tile_pool(name="fin_sb", bufs=2),
        )
        with pools[0] as pp_ps, pools[1] as score_ps, pools[2] as oaug_ps, \
                pools[3] as rope_sb, pools[4] as exp_sb, pools[5] as norm_sb, \
                pools[6] as fin_sb:

            def _proj_qk_fillers(t, j):
                """Pair-1 projection as a list of emission closures: the
                matmuls interleave into attn0's i-loop, filling the PE's
                exp-wait gaps (the i-loops run ~15% ACT-bound)."""
                csl = slice(j * NQ, (j + 1) * NQ)
                state = {}

                def _mm(qk, k, w_sb):
                    def go():
                        if qk not in state:
                            state[qk] = pp_ps.tile([P, NQ], F32,
                                                   name="projq_ps" if qk == 0
                                                   else "projk_ps", tag="pp")
                        nc.tensor.matmul(
                            state[qk][:],
                            lhsT=w_sb[:, k, t * P:(t + 1) * P],
                            rhs=xt_sb[:, k, j % 2, :],
                            start=(k == 0), stop=(k == KT - 1))
                    return go

                def _chain(qk, dst):
                    def go():
                        ps = state[qk][:]
                        shuf = rope_sb.tile([P, NQ], F32, name="shuf", tag="rope")
                        t1 = rope_sb.tile([P, NQ], F32, name="rope_a", tag="rope")
                        t2 = rope_sb.tile([P, NQ], F32, name="rope_b", tag="rope")
                        nc.vector.stream_shuffle(shuf[:], ps, SHUF_MASK)
                        nc.vector.tensor_mul(t1[:], ps, cos_sb[:, csl])
                        nc.gpsimd.tensor_mul(t2[:], shuf[:], sin_sb[:, csl])
                        nc.vector.tensor_add(dst[:, t, csl], t1[:], t2[:])
                    return go

                out = []
                for qk, w_sb, dst in ((0, wq_sb, qtr_sb), (1, wk_sb, ktr_sb)):
                    for k in range(KT):
                        out.append(_mm(qk, k, w_sb))
                    out.append(_chain(qk, dst))
                return out

            def _proj_qk(t, j):
                """q+k projection + rope for head pair t, chunk j.

                Pair 0 (emitted while the score pool is idle) interleaves the
                q/k k-loops in one paired score-pool tile — matching the
                prologue DMA triplet pacing. Pair 1 is emitted during attn0's
                score cycling, so it uses sequential k-loops on single-bank
                pp tiles instead of stealing a score buffer."""
                csl = slice(j * NQ, (j + 1) * NQ)
                if t == 0:
                    # sequential q-then-k loops in one paired score-pool tile
                    # (idle during projections): the q loop depends only on
                    # wq+xt, so the prologue feed keeps it PE-bound while wk
                    # streams in behind it
                    ps2 = score_ps.tile([P, 2, NQ], F32, name="proj_ps", tag="sc")
                    pss = (ps2[:, 0, :], ps2[:, 1, :])
                    for qk, w_sb in ((0, wq_sb), (1, wk_sb)):
                        for k in range(KT):
                            nc.tensor.matmul(
                                pss[qk],
                                lhsT=w_sb[:, k, t * P:(t + 1) * P],
                                rhs=xt_sb[:, k, j % 2, :],
                                start=(k == 0), stop=(k == KT - 1))
                else:
                    psq = pp_ps.tile([P, NQ], F32, name="projq_ps", tag="pp")
                    psk = pp_ps.tile([P, NQ], F32, name="projk_ps", tag="pp")
                    pss = (psq[:], psk[:])
                    for qk, w_sb in ((0, wq_sb), (1, wk_sb)):
                        for k in range(KT):
                            nc.tensor.matmul(
                                pss[qk],
                                lhsT=w_sb[:, k, t * P:(t + 1) * P],
                                rhs=xt_sb[:, k, j % 2, :],
                                start=(k == 0), stop=(k == KT - 1))
                for qk, dst in ((0, qtr_sb), (1, ktr_sb)):
                    ps = pss[qk]
                    shuf = rope_sb.tile([P, NQ], F32, name="shuf", tag="rope")
                    t1 = rope_sb.tile([P, NQ], F32, name="rope_a", tag="rope")
                    t2 = rope_sb.tile([P, NQ], F32, name="rope_b", tag="rope")
                    nc.vector.stream_shuffle(shuf[:], ps, SHUF_MASK)
                    nc.vector.tensor_mul(t1[:], ps, cos_sb[:, csl])
                    nc.gpsimd.tensor_mul(t2[:], shuf[:], sin_sb[:, csl])
                    nc.vector.tensor_add(dst[:, t, csl], t1[:], t2[:])

            def _proj_v(st, on_dve=False):
                psv = pp_ps.tile([P, NH * DK], F32, name="projv_ps", tag="pp")
                for k in range(KT):
                    nc.tensor.matmul(
                        psv[:],
                        lhsT=xt_sb[:, k, (st // 4) % 2, (st % 4) * P:(st % 4 + 1) * P],
                        rhs=wv_sb[:, k, :],
                        start=(k == 0), stop=(k == KT - 1))
                dst = vaug_sb[:, st, :, 0:DK]
                src = psv[:].rearrange("p (h c) -> p h c", h=NH)
                if on_dve:
                    nc.vector.tensor_copy(dst, src)
                else:
                    nc.scalar.copy(dst, src)

            def _attn(pr, j, fillers=None):
                """Causal attention for head pair pr, query chunk j."""
                imax = 4 * j + 3
                fillers = list(fillers or [])
                fidx = [0]

                def _fill(n):
                    while n > 0 and fidx[0] < len(fillers):
                        fillers[fidx[0]]()
                        fidx[0] += 1
                        n -= 1
                nsl = slice(j * NQ, (j + 1) * NQ)
                oa = [oaug_ps.tile([DK + 1, NQ], F32, name="oaug") for _ in range(2)]

                def _scores(i, start, stop):
                    # diagonal blocks (offset d = i - 4j): columns below
                    # 128*d are fully causal-masked — compute only [c0:512).
                    # Exception: d=3 would give N=128, which fp32r runs at a
                    # 4x penalty (ap_size < 256); widening it to N=256 halves
                    # its real cost — the extra columns are fully masked and
                    # contribute exact zeros to the attn@v accumulation
                    d = max(i - 4 * j, 0)
                    c0 = 2 * P if d == 3 else P * d
                    w = NQ - c0
                    sc = score_ps.tile([P, 2, NQ], F32, name="score", tag="sc")
                    for hh in range(2):
                        hb = hh * DK
                        nc.tensor.matmul(
                            sc[:, hh, 0:w],
                            lhsT=ktr_sb[hb:hb + DK, pr, i * P:(i + 1) * P],
                            rhs=qtr_sb[hb:hb + DK, pr,
                                       j * NQ + c0:(j + 1) * NQ],
                            start=True, stop=True)
                    eb = exp_sb.tile([P, 2, NQ], F32R, name="expblk")
                    nc.scalar.activation(
                        eb[:, :, 0:w], sc[:, :, 0:w],
                        mybir.ActivationFunctionType.Exp,
                        scale=float(1.0 / np.sqrt(DK)))
                    if i >= 4 * j:   # diagonal block: causal mask, both heads
                        # masked cells satisfy col - base < p <= 127: only
                        # the first 128+(-base) columns can ever be zeroed —
                        # restricting the AP there cuts the Pool op (and the
                        # chain the attn@v waits on); later columns are
                        # already correct in SBUF
                        base = -P if d == 3 else 0
                        mw = min(w, P - base)
                        nc.gpsimd.affine_select(
                            out=eb[:, :, 0:mw], in_=eb[:, :, 0:mw],
                            compare_op=mybir.AluOpType.is_ge,
                            fill=0.0,
                            base=base,
                            channel_multiplier=-1,
                            pattern=[[0, 2], [1, mw]])
                    return (eb, c0, w, start, stop)

                def _attnv(i, blk):
                    eb, c0, w, start, stop = blk
                    for hh in range(2):
                        nc.tensor.matmul(
                            oa[hh][:, c0:NQ],
                            lhsT=vaug_sb[:, i, 2 * pr + hh, :],
                            rhs=eb[:, hh, 0:w],
                            start=start, stop=stop)

                # diagonal blocks first: their exp->mask chains are the
                # longest, so the drain attn@v's at the end consume mask-free
                # full blocks instead of waiting on fresh masks
                i_order = list(range(4 * j, imax + 1)) + list(range(0, 4 * j))
                nsteps = imax + 1 + min(LAG, imax + 1)
                per_step = -(-len(fillers) // max(nsteps, 1))
                pending = {}
                for pos, i in enumerate(i_order):
                    pending[i] = _scores(i, start=(pos == 0),
                                         stop=(pos == imax))
                    _fill(per_step)
                    if pos >= LAG:
                        ii = i_order[pos - LAG]
                        _attnv(ii, pending.pop(ii))
                for ii in i_order[max(0, imax + 1 - LAG):]:
                    _attnv(ii, pending.pop(ii))
                    _fill(per_step)
                _fill(len(fillers))   # flush any remainder

                return (oa,)

            def _recips(oa):
                rcs = []
                for hh in (1, 0):
                    rc = norm_sb.tile([DK + 1, NQ], F32R, name="recip", tag="nrm")
                    nc.vector.reciprocal(rc[DK:DK + 1, :], oa[hh][DK:DK + 1, :])
                    rcs.append((hh, rc))
                return rcs

            def _norm_b(pr, j, oa, tail=False, rcs=None):
                """Normalize: reciprocal of the denominator row (row 64, the
                ones column of v), broadcast, multiply into outt. Emitted
                behind filler matmul work so the PE never waits the chain."""
                nsl = slice(j * NQ, (j + 1) * NQ)
                if rcs is None:
                    rcs = _recips(oa)
                bss = []
                for hh, rc in rcs:
                    bc = pp_ps.tile([DK, NQ], F32, name="bcast_ps", tag="pp")
                    nc.tensor.matmul(
                        bc[:],
                        lhsT=ones_sb[DK:DK + 1, :],
                        rhs=rc[DK:DK + 1, :],
                        start=True, stop=True)
                    bs = norm_sb.tile([DK, NQ], F32, name="bcast_sb", tag="nrm")
                    if tail:
                        # final chain: ACT is idle, shorten the DVE span
                        nc.scalar.copy(bs[:], bc[:])
                    else:
                        nc.vector.tensor_copy(bs[:], bc[:])
                    bss.append((hh, bs))
                for hh, bs in bss:
                    if hh == 0:
                        nc.vector.tensor_mul(
                            outt_sb[0:DK, pr, nsl], oa[hh][0:DK, :], bs[:])
                    else:
                        # odd head: normalize at base 0 (DVE cannot cross
                        # partitions), then gpsimd places rows 64-127
                        ot_tmp = norm_sb.tile([DK, NQ], BF16, name="ot_tmp",
                                              tag="nrm")
                        nc.vector.tensor_mul(ot_tmp[:], oa[hh][0:DK, :], bs[:])
                        nc.gpsimd.tensor_copy(outt_sb[DK:P, pr, nsl], ot_tmp[:])

            def _wo_col_fillers(j):
                """wo(j) as emission closures interleaved into attn1(j+1)."""
                nsl = slice(j * NQ, (j + 1) * NQ)
                half = D // P // 2
                state = {}

                def _one(ot):
                    def go():
                        if 'fin' not in state:
                            state['fin'] = fin_sb.tile([P, D // P, NQ], BF16,
                                                       name="fin")
                        fin = state['fin']
                        ps = pp_ps.tile([P, NQ], F32, name="wo_ps", tag="pp")
                        for t in range(OT):
                            nc.tensor.matmul(
                                ps[:],
                                lhsT=wo_sb[:, t, ot * P:(ot + 1) * P],
                                rhs=outt_sb[:, t, nsl],
                                start=(t == 0), stop=(t == OT - 1))
                        if ot % 2 == 0:
                            nc.vector.tensor_copy(fin[:, ot, :], ps[:])
                        else:
                            nc.scalar.copy(fin[:, ot, :], ps[:])
                        if (ot + 1) % half == 0:
                            lo = ot + 1 - half
                            nc.sync.dma_start(
                                yt[lo * P:(ot + 1) * P, nsl].rearrange(
                                    "(a p) s -> p a s", a=half),
                                fin[:, lo:ot + 1, :])
                    return go

                return [_one(ot) for ot in range(D // P)]

            def _wo_col(j, tail=False):
                nsl = slice(j * NQ, (j + 1) * NQ)
                half = D // P // 2
                fin = fin_sb.tile([P, D // P, NQ], BF16, name="fin")
                for ot in range(D // P):
                    if tail and ot % 2 == 1:
                        # attention is over: borrow the idle score pool so
                        # two output chunks are in flight
                        ps2 = score_ps.tile([P, 2, NQ], F32, name="wo_ps2",
                                            tag="sc")
                        ps = ps2[:, 0, :]
                    else:
                        ps = pp_ps.tile([P, NQ], F32, name="wo_ps", tag="pp")
                    for t in range(OT):
                        nc.tensor.matmul(
                            ps[:],
                            lhsT=wo_sb[:, t, ot * P:(ot + 1) * P],
                            rhs=outt_sb[:, t, nsl],
                            start=(t == 0), stop=(t == OT - 1))
                    # alternate engines so consecutive PSUM drains pipeline
                    if ot % 2 == 0:
                        nc.vector.tensor_copy(fin[:, ot, :], ps[:])
                    else:
                        nc.scalar.copy(fin[:, ot, :], ps[:])
                    step = 2 if tail else half
                    if (ot + 1) % step == 0:
                        # batched writeback: one DMA issue beats per-chunk
                        # 700ns-spaced issues; the tail uses quarters so the
                        # last transfer trails the last drain minimally
                        lo = ot + 1 - step
                        nc.sync.dma_start(
                            yt[lo * P:(ot + 1) * P, nsl].rearrange(
                                "(a p) s -> p a s", a=step),
                            fin[:, lo:ot + 1, :])

            for j in range(NT):
                # prefetch chunk j+1 while computing chunk j
                if j + 1 < NT:
                    jsl = slice((j + 1) * NQ, (j + 2) * NQ)
                    nc.sync.dma_start(
                        xt_sb[:, :, (j + 1) % 2, :],
                        xt[:, jsl].rearrange("(k p) s -> p k s", k=KT))
                    nc.sync.dma_start(cos_sb[:, jsl], cosr[:, jsl])
                    nc.sync.dma_start(sin_sb[:, jsl], sinr[:, jsl])
                if j == 0:
                    nc.sync.dma_start(wo_sb[:, :, :],
                                      wo[:, :, :].rearrange("t p d -> p t d"))
                # pair 0's q/k first: its rope chain gates the first scores;
                # pair 1's projection is deferred past attn0 so its rope
                # chain (DVE/Pool) doesn't queue ahead of attn0's masks
                _proj_qk(0, j)
                for st in range(4 * j, 4 * j + 4):
                    _proj_v(st)
                if j > 0:
                    _norm_b(1, j - 1, *saved)   # recips long done: no stall
                st0 = _attn(0, j, fillers=_proj_qk_fillers(1, j))
                if j > 0:
                    _wo_col(j - 1)   # fills PE while pair 0's recips run
                _norm_b(0, j, *st0)
                saved = _attn(1, j)
                if j == NT - 1:
                    tail_rcs = _recips(saved[0])
            _norm_b(1, NT - 1, *saved, tail=True, rcs=tail_rcs)
            _wo_col(NT - 1, tail=True)

    nc.compile()
    return nc


_NC_CACHE = {}


def _get_nc():
    if "nc" not in _NC_CACHE:
        _NC_CACHE["nc"] = build_nc()
    return _NC_CACHE["nc"]


_HALF = DK // 2
# 16-deep interleave: quadrant q holds freqs 16q..16q+15, x1 rows then x2 rows
_PERM = np.array([2 * (16 * q + i) + c
                  for q in (0, 1) for c in (0, 1) for i in range(16)])


BF16NP = ml_dtypes.bfloat16


def _prep_qk(W, heads):
    """Per-head RoPE-permuted projection weights (transposed for lhsT)."""
    Wh = W.reshape(H, DK, D)[heads][:, _PERM, :]                    # [NH, DK, D]
    return np.ascontiguousarray(Wh.reshape(NH * DK, D).T.astype(BF16NP))


def make_in_maps(x, token_positions, Wq, Wk, Wv, Wo):
    x = np.asarray(x, dtype=np.float32)
    Wq = np.asarray(Wq, dtype=np.float32)
    Wk = np.asarray(Wk, dtype=np.float32)
    Wv = np.asarray(Wv, dtype=np.float32)
    Wo = np.asarray(Wo, dtype=np.float32)
    pos = np.asarray(token_positions)

    j = np.arange(_HALF, dtype=np.float64)
    inv_freq = ROPE_THETA ** (-2.0 * j / DK)                        # [32]

    in_maps = []
    for core in range(NCORES):
        b = core // GROUPS
        g = core % GROUPS
        heads = list(range(g * NH, (g + 1) * NH))
        wq_ = _prep_qk(Wq, heads)
        wk_ = _prep_qk(Wk, heads)
        wv_ = np.ascontiguousarray(Wv.reshape(H, DK, D)[heads].reshape(NH * DK, D).T.astype(BF16NP))
        wo_ = np.ascontiguousarray(Wo.T.reshape(H, DK, D)[heads].reshape(OT, P, D).astype(BF16NP))
        ang = np.outer(inv_freq, pos[b].astype(np.float64))          # [32, S]
        cos32 = np.cos(ang)
        sin32 = np.sin(ang)
        # row layout per 64-block: [f0..15, f0..15, f16..31, f16..31]
        cos64 = np.concatenate([cos32[0:16], cos32[0:16],
                                cos32[16:32], cos32[16:32]])
        sin64 = np.concatenate([-sin32[0:16], sin32[0:16],
                                -sin32[16:32], sin32[16:32]])
        cosr = np.tile(cos64, (2, 1)).astype(np.float32)             # [128, S]
        sinr = np.tile(sin64, (2, 1)).astype(np.float32)
        in_maps.append({
            "xt": np.ascontiguousarray(x[b].T.astype(BF16NP)),
            "wq": wq_, "wk": wk_, "wv": wv_, "wo": wo_,
            "cosr": cosr, "sinr": sinr,
            "ones": np.ones((P, DK), dtype=np.float32),
        })
    return in_maps


def _gather(results):
    outs = [np.asarray(r["yt"], dtype=np.float32) for r in results]
    y = np.stack([
        sum(outs[b * GROUPS + 1: (b + 1) * GROUPS], outs[b * GROUPS]).T
        for b in range(B)
    ])
    return np.ascontiguousarray(y)


def kernel(x, token_positions, Wq, Wk, Wv, Wo):
    in_maps = make_in_maps(x, token_positions, Wq, Wk, Wv, Wo)
    res = run_bass_kernel_spmd(_get_nc(), in_maps, core_ids=list(range(NCORES)))
    return _gather(res.results)


def kernel_traced(x, token_positions, Wq, Wk, Wv, Wo, **kwargs):
    """Like kernel() but with NTFF tracing; returns (output, BassKernelResults)."""
    in_maps = make_in_maps(x, token_positions, Wq, Wk, Wv, Wo)
    res = run_bass_kernel_spmd(_get_nc(), in_maps, core_ids=list(range(NCORES)),
                               trace=True, **kwargs)
    return _gather(res.results), res


# revision 81
# speedup vs baseline: 1.0495x; 1.0030x over previous
"""Multi-head self-attention (RoPE, causal softmax) — Trainium2 Bass kernel.

Sharding over 8 NeuronCores: batch (2) x head-groups (16 heads / 4 groups).
Each core handles one batch element and 4 heads. Single software-pipelined
phase, streamed per query/key column block j (512 positions):

  - x / weights / cos/sin stream in as few batched multi-tile DMAs (the
    565ns-per-DMA descriptor-issue rate, not bytes, paces the prologue);
    x and the weights travel in bf16, halving prologue bytes; the first
    projection matmul starts ~3us in instead of ~36us
  - q/k projections for chunk j (head pairs stacked on partitions), RoPE
    via DVE stream_shuffle: head dims are host-permuted into a 16-deep
    interleave (x1/x2 component blocks of 16 rows inside each 32-partition
    quadrant) so the rotation partner is a within-quadrant 16-row swap —
    no second "rotated weights" matmul stream on the PE
  - v projection for key blocks 4j..4j+3 into v_aug (extra ones-column so
    the softmax denominator falls out of the attn@v matmul, PSUM row 64)
  - causal attention for column j, both head pairs: scoresT[m,n] blocks
    on PE writing a paired PSUM tile [128, 2, w], exp on ACT (scale=1/8
    fused) over both heads at once, causal mask via one gpsimd
    affine_select (pattern [[0,2],[1,w]]), attn@v per head; diagonal
    (masked) blocks are computed first so the end-of-loop drain consumes
    mask-free full blocks; scores run LAG blocks ahead of their attn@v
    consumers so the in-order PE never stalls on the exp->mask chain
  - per-head normalize, split in two stages: (A) DVE reciprocals of the
    denominator rows right after the i-loop, (B) ones-column PE broadcast
    matmul + drain + multiplies, deferred until fresh matmul work (next
    chunk's projections, wo) covers the latency; the odd head's rows reach
    partitions 64-127 via a gpsimd cross-partition copy (gpsimd may not
    touch PSUM, DVE may not cross partitions; this split satisfies both)
  - output projection wo(j-1) is emitted between attn pair 0 and pair 1 of
    chunk j; the final wo borrows the idle score-pool PSUM to run two
    output chunks in flight; partials are written to DRAM in bf16 via two
    batched half-column DMAs per chunk (host accumulates in fp32)

Attention matmuls run in float32r (full rate on the PE array); the x/W
projections run in bf16. Measured rel err ~3.7e-3 vs the fp32 reference.
"""

from contextlib import ExitStack

import ml_dtypes
import numpy as np

import concourse.bass as bass
import concourse.bacc as bacc
import concourse.tile as tile
from concourse import mybir
from concourse.bass_utils import run_bass_kernel_spmd

# problem shape (hardcoded: graded standalone)
B, S, D, H, DK = 2, 2048, 1024, 16, 64
NCORES = 8
GROUPS = NCORES // B  # 4 head-groups (cores) per batch element
NH = H // GROUPS      # 4 heads per core
ROPE_THETA = 10000.0

P = 128
NQ = 512              # query-block (matmul moving free dim)
NT = S // NQ          # 4 query blocks
NM = S // P           # 16 key blocks
KT = D // P           # 8 contraction tiles for the x-projections
OT = NH * DK // P     # 2 stacked head-pair tiles for q/k
LAG = 8               # score blocks emitted ahead of attn@v consumers

F32 = mybir.dt.float32
F32R = mybir.dt.float32r
BF16 = mybir.dt.bfloat16

# swap the two 16-row component blocks inside each 32-partition quadrant
SHUF_MASK = list(range(16, 32)) + list(range(0, 16))


def build_nc():
    nc = bacc.Bacc("TRN2", target_bir_lowering=False, debug=False)

    xt = nc.dram_tensor("xt", [D, S], BF16, kind="ExternalInput")        # x[b].T
    wq = nc.dram_tensor("wq", [D, NH * DK], BF16, kind="ExternalInput")  # perm'd, T
    wk = nc.dram_tensor("wk", [D, NH * DK], BF16, kind="ExternalInput")
    wv = nc.dram_tensor("wv", [D, NH * DK], BF16, kind="ExternalInput")
    wo = nc.dram_tensor("wo", [OT, P, D], BF16, kind="ExternalInput")   # pair-stacked
    cosr = nc.dram_tensor("cosr", [P, S], F32, kind="ExternalInput")
    sinr = nc.dram_tensor("sinr", [P, S], F32, kind="ExternalInput")    # sign-folded
    yt = nc.dram_tensor("yt", [D, S], BF16, kind="ExternalOutput")      # partial y.T
    ones_dram = nc.dram_tensor("ones", [P, DK], F32R, kind="ExternalInput")

    with tile.TileContext(nc) as tc, ExitStack() as ctx, \
            nc.allow_low_precision(reason="float32r matmul inputs and bf16 partial outputs are within tolerance"):
        # ---- persistent SBUF (whole kernel) ----
        persist = ctx.enter_context(tc.tile_pool(name="persist", bufs=1))
        cos_sb = persist.tile([P, S], F32, name="cos_sb")
        sin_sb = persist.tile([P, S], F32, name="sin_sb")
        ones_sb = persist.tile([P, DK], F32R, name="ones_sb")
        qtr_sb = persist.tile([P, OT, S], F32R, name="qtr_sb")    # rope(q)^T pairs
        ktr_sb = persist.tile([P, OT, S], F32R, name="ktr_sb")
        vaug_sb = persist.tile([P, NM, NH, DK + 1], F32R, name="vaug_sb")
        outt_sb = persist.tile([P, OT, S], BF16, name="outt_sb")  # out^T pairs
        wq_sb = persist.tile([P, KT, NH * DK], BF16, name="wq_sb")
        wk_sb = persist.tile([P, KT, NH * DK], BF16, name="wk_sb")
        wv_sb = persist.tile([P, KT, NH * DK], BF16, name="wv_sb")
        wo_sb = persist.tile([P, OT, D], BF16, name="wo_sb")
        xt_sb = persist.tile([P, KT, 2, NQ], BF16, name="xt_sb")  # j-chunk double buffer

        # prologue DMAs: batched multi-tile transfers — the SP queue's
        # 565ns/DMA issue rate, not bytes, paces the prologue, so one
        # 3-dim-AP DMA per tensor beats eight per-tile DMAs. xt rides the
        # scalar queue so both descriptor generators run in parallel.
        nc.sync.dma_start(wq_sb[:, 0:2, :],
                          wq[0:2 * P, :].rearrange("(k p) c -> p k c", k=2))
        nc.sync.dma_start(wq_sb[:, 2:KT // 2, :],
                          wq[2 * P:D // 2, :].rearrange("(k p) c -> p k c", k=2))
        nc.sync.dma_start(wq_sb[:, KT // 2:, :],
                          wq[D // 2:, :].rearrange("(k p) c -> p k c", k=KT // 2))
        nc.sync.dma_start(wk_sb[:, 0:KT // 2, :],
                          wk[0:D // 2, :].rearrange("(k p) c -> p k c", k=KT // 2))
        nc.sync.dma_start(wk_sb[:, KT // 2:, :],
                          wk[D // 2:, :].rearrange("(k p) c -> p k c", k=KT // 2))
        nc.scalar.dma_start(xt_sb[:, 0:2, 0, :],
                            xt[0:2 * P, 0:NQ].rearrange("(k p) s -> p k s", k=2))
        nc.scalar.dma_start(xt_sb[:, 2:KT // 2, 0, :],
                            xt[2 * P:D // 2, 0:NQ].rearrange("(k p) s -> p k s", k=2))
        nc.scalar.dma_start(xt_sb[:, KT // 2:, 0, :],
                            xt[D // 2:, 0:NQ].rearrange("(k p) s -> p k s", k=KT // 2))
        nc.sync.dma_start(cos_sb[:, 0:NQ], cosr[:, 0:NQ])
        nc.sync.dma_start(sin_sb[:, 0:NQ], sinr[:, 0:NQ])
        nc.sync.dma_start(wv_sb[:, :, :], wv[:, :].rearrange("(k p) c -> p k c", k=KT))
        # ones for the denominator broadcast; the scattered v_aug ones
        # column is filled by a cheap DVE copy (a strided DMA costs ~3.6us)
        nc.sync.dma_start(ones_sb[:], ones_dram[:, :])
        nc.vector.tensor_copy(vaug_sb[:, :, :, DK],
                              ones_sb[:].rearrange("p (a b) -> p a b", a=NM))

        pools = (
            tc.tile_pool(name="pp_ps", bufs=2, space="PSUM"),      # proj/v/bcast/wo
            tc.tile_pool(name="score_ps", bufs=2, space="PSUM"),   # paired scores
            tc.tile_pool(name="oaug_ps", bufs=2, space="PSUM"),
            tc.tile_pool(name="rope_sb", bufs=9),
            tc.tile_pool(name="exp_sb", bufs=11),
            tc.tile_pool(name="norm_sb", bufs=12),
            tc.tile_pool(name="fin_sb", bufs=2),
        )
        with pools[0] as pp_ps, pools[1] as score_ps, pools[2] as oaug_ps, \
                pools[3] as rope_sb, pools[4] as exp_sb, pools[5] as norm_sb, \
                pools[6] as fin_sb:

            def _proj_qk_fillers(t, j):
                """Pair-1 projection as a list of emission closures: the
                matmuls interleave into attn0's i-loop, filling the PE's
                exp-wait gaps (the i-loops run ~15% ACT-bound)."""
                csl = slice(j * NQ, (j + 1) * NQ)
                state = {}

                def _mm(qk, k, w_sb):
                    def go():
                        if qk not in state:
                            state[qk] = pp_ps.tile([P, NQ], F32,
                                                   name="projq_ps" if qk == 0
                                                   else "projk_ps", tag="pp")
                        nc.tensor.matmul(
                            state[qk][:],
                            lhsT=w_sb[:, k, t * P:(t + 1) * P],
                            rhs=xt_sb[:, k, j % 2, :],
                            start=(k == 0), stop=(k == KT - 1))
                    return go

                def _chain(qk, dst):
                    def go():
                        ps = state[qk][:]
                        shuf = rope_sb.tile([P, NQ], F32, name="shuf", tag="rope")
                        t1 = rope_sb.tile([P, NQ], F32, name="rope_a", tag="rope")
                        t2 = rope_sb.tile([P, NQ], F32, name="rope_b", tag="rope")
                        nc.vector.stream_shuffle(shuf[:], ps, SHUF_MASK)
                        nc.vector.tensor_mul(t1[:], ps, cos_sb[:, csl])
                        nc.gpsimd.tensor_mul(t2[:], shuf[:], sin_sb[:, csl])
                        nc.vector.tensor_add(dst[:, t, csl], t1[:], t2[:])
                    return go

                out = []
                for qk, w_sb, dst in ((0, wq_sb, qtr_sb), (1, wk_sb, ktr_sb)):
                    for k in range(KT):
                        out.append(_mm(qk, k, w_sb))
                    out.append(_chain(qk, dst))
                return out

            def _proj_qk(t, j):
                """q+k projection + rope for head pair t, chunk j.

                Pair 0 (emitted while the score pool is idle) interleaves the
                q/k k-loops in one paired score-pool tile — matching the
                prologue DMA triplet pacing. Pair 1 is emitted during attn0's
                score cycling, so it uses sequential k-loops on single-bank
                pp tiles instead of stealing a score buffer."""
                csl = slice(j * NQ, (j + 1) * NQ)
                if t == 0:
                    # sequential q-then-k loops in one paired score-pool tile
                    # (idle during projections): the q loop depends only on
                    # wq+xt, so the prologue feed keeps it PE-bound while wk
                    # streams in behind it
                    ps2 = score_ps.tile([P, 2, NQ], F32, name="proj_ps", tag="sc")
                    pss = (ps2[:, 0, :], ps2[:, 1, :])
                    for qk, w_sb in ((0, wq_sb), (1, wk_sb)):
                        for k in range(KT):
                            nc.tensor.matmul(
                                pss[qk],
                                lhsT=w_sb[:, k, t * P:(t + 1) * P],
                                rhs=xt_sb[:, k, j % 2, :],
                                start=(k == 0), stop=(k == KT - 1))
                else:
                    psq = pp_ps.tile([P, NQ], F32, name="projq_ps", tag="pp")
                    psk = pp_ps.tile([P, NQ], F32, name="projk_ps", tag="pp")
                    pss = (psq[:], psk[:])
                    for qk, w_sb in ((0, wq_sb), (1, wk_sb)):
                        for k in range(KT):
                            nc.tensor.matmul(
                                pss[qk],
                                lhsT=w_sb[:, k, t * P:(t + 1) * P],
                                rhs=xt_sb[:, k, j % 2, :],
                                start=(k == 0), stop=(k == KT - 1))
                for qk, dst in ((0, qtr_sb), (1, ktr_sb)):
                    ps = pss[qk]
                    shuf = rope_sb.tile([P, NQ], F32, name="shuf", tag="rope")
                    t1 = rope_sb.tile([P, NQ], F32, name="rope_a", tag="rope")
                    t2 = rope_sb.tile([P, NQ], F32, name="rope_b", tag="rope")
                    nc.vector.stream_shuffle(shuf[:], ps, SHUF_MASK)
                    nc.vector.tensor_mul(t1[:], ps, cos_sb[:, csl])
                    nc.gpsimd.tensor_mul(t2[:], shuf[:], sin_sb[:, csl])
                    nc.vector.tensor_add(dst[:, t, csl], t1[:], t2[:])

            def _proj_v(st, on_dve=False):
                psv = pp_ps.tile([P, NH * DK], F32, name="projv_ps", tag="pp")
                for k in range(KT):
                    nc.tensor.matmul(
                        psv[:],
                        lhsT=xt_sb[:, k, (st // 4) % 2, (st % 4) * P:(st % 4 + 1) * P],
                        rhs=wv_sb[:, k, :],
                        start=(k == 0), stop=(k == KT - 1))
                dst = vaug_sb[:, st, :, 0:DK]
                src = psv[:].rearrange("p (h c) -> p h c", h=NH)
                if on_dve:
                    nc.vector.tensor_copy(dst, src)
                else:
                    nc.scalar.copy(dst, src)

            def _attn(pr, j, fillers=None):
                """Causal attention for head pair pr, query chunk j."""
                imax = 4 * j + 3
                fillers = list(fillers or [])
                fidx = [0]

                def _fill(n):
                    while n > 0 and fidx[0] < len(fillers):
                        fillers[fidx[0]]()
                        fidx[0] += 1
                        n -= 1
                nsl = slice(j * NQ, (j + 1) * NQ)
                oa = [oaug_ps.tile([DK + 1, NQ], F32, name="oaug") for _ in range(2)]

                def _scores(i, start, stop):
                    # diagonal blocks (offset d = i - 4j): columns below
                    # 128*d are fully causal-masked — compute only [c0:512).
                    # Exception: d=3 would give N=128, which fp32r runs at a
                    # 4x penalty (ap_size < 256); widening it to N=256 halves
                    # its real cost — the extra columns are fully masked and
                    # contribute exact zeros to the attn@v accumulation
                    d = max(i - 4 * j, 0)
                    c0 = 2 * P if d == 3 else P * d
                    w = NQ - c0
                    sc = score_ps.tile([P, 2, NQ], F32, name="score", tag="sc")
                    for hh in range(2):
                        hb = hh * DK
                        nc.tensor.matmul(
                            sc[:, hh, 0:w],
                            lhsT=ktr_sb[hb:hb + DK, pr, i * P:(i + 1) * P],
                            rhs=qtr_sb[hb:hb + DK, pr,
                                       j * NQ + c0:(j + 1) * NQ],
                            start=True, stop=True)
                    eb = exp_sb.tile([P, 2, NQ], F32R, name="expblk")
                    nc.scalar.activation(
                        eb[:, :, 0:w], sc[:, :, 0:w],
                        mybir.ActivationFunctionType.Exp,
                        scale=float(1.0 / np.sqrt(DK)))
                    if i >= 4 * j:   # diagonal block: causal mask, both heads
                        # masked cells satisfy col - base < p <= 127: only
                        # the first 128+(-base) columns can ever be zeroed —
                        # restricting the AP there cuts the Pool op (and the
                        # chain the attn@v waits on); later columns are
                        # already correct in SBUF
                        base = -P if d == 3 else 0
                        mw = min(w, P - base)
                        nc.gpsimd.affine_select(
                            out=eb[:, :, 0:mw], in_=eb[:, :, 0:mw],
                            compare_op=mybir.AluOpType.is_ge,
                            fill=0.0,
                            base=base,
                            channel_multiplier=-1,
                            pattern=[[0, 2], [1, mw]])
                    return (eb, c0, w, start, stop)

                def _attnv(i, blk):
                    eb, c0, w, start, stop = blk
                    for hh in range(2):
                        nc.tensor.matmul(
                            oa[hh][:, c0:NQ],
                            lhsT=vaug_sb[:, i, 2 * pr + hh, :],
                            rhs=eb[:, hh, 0:w],
                            start=start, stop=stop)

                # diagonal blocks first: their exp->mask chains are the
                # longest, so the drain attn@v's at the end consume mask-free
                # full blocks instead of waiting on fresh masks
                i_order = list(range(4 * j, imax + 1)) + list(range(0, 4 * j))
                nsteps = imax + 1 + min(LAG, imax + 1)
                per_step = -(-len(fillers) // max(nsteps, 1))
                pending = {}
                for pos, i in enumerate(i_order):
                    pending[i] = _scores(i, start=(pos == 0),
                                         stop=(pos == imax))
                    _fill(per_step)
                    if pos >= LAG:
                        ii = i_order[pos - LAG]
                        _attnv(ii, pending.pop(ii))
                for ii in i_order[max(0, imax + 1 - LAG):]:
                    _attnv(ii, pending.pop(ii))
                    _fill(per_step)
                _fill(len(fillers))   # flush any remainder

                return (oa,)

            def _recips(oa):
                rcs = []
                for hh in (1, 0):
                    rc = norm_sb.tile([DK + 1, NQ], F32R, name="recip", tag="nrm")
                    nc.vector.reciprocal(rc[DK:DK + 1, :], oa[hh][DK:DK + 1, :])
                    rcs.append((hh, rc))
                return rcs

            def _norm_b(pr, j, oa, tail=False, rcs=None):
                """Normalize: reciprocal of the denominator row (row 64, the
                ones column of v), broadcast, multiply into outt. Emitted
                behind filler matmul work so the PE never waits the chain."""
                nsl = slice(j * NQ, (j + 1) * NQ)
                if rcs is None:
                    rcs = _recips(oa)
                bss = []
                for hh, rc in rcs:
                    bc = pp_ps.tile([DK, NQ], F32, name="bcast_ps", tag="pp")
                    nc.tensor.matmul(
                        bc[:],
                        lhsT=ones_sb[DK:DK + 1, :],
                        rhs=rc[DK:DK + 1, :],
                        start=True, stop=True)
                    bs = norm_sb.tile([DK, NQ], F32, name="bcast_sb", tag="nrm")
                    if tail:
                        # final chain: ACT is idle, shorten the DVE span
                        nc.scalar.copy(bs[:], bc[:])
                    else:
                        nc.vector.tensor_copy(bs[:], bc[:])
                    bss.append((hh, bs))
                for hh, bs in bss:
                    if hh == 0:
                        nc.vector.tensor_mul(
                            outt_sb[0:DK, pr, nsl], oa[hh][0:DK, :], bs[:])
                    else:
                        # odd head: normalize at base 0 (DVE cannot cross
                        # partitions), then gpsimd places rows 64-127
                        ot_tmp = norm_sb.tile([DK, NQ], BF16, name="ot_tmp",
                                              tag="nrm")
                        nc.vector.tensor_mul(ot_tmp[:], oa[hh][0:DK, :], bs[:])
                        nc.gpsimd.tensor_copy(outt_sb[DK:P, pr, nsl], ot_tmp[:])

            def _wo_col_fillers(j):
                """wo(j) as emission closures interleaved into attn1(j+1)."""
                nsl = slice(j * NQ, (j + 1) * NQ)
                half = D // P // 2
                state = {}

                def _one(ot):
                    def go():
                        if 'fin' not in state:
                            state['fin'] = fin_sb.tile([P, D // P, NQ], BF16,
                                                       name="fin")
                        fin = state['fin']
                        ps = pp_ps.tile([P, NQ], F32, name="wo_ps", tag="pp")
                        for t in range(OT):
                            nc.tensor.matmul(
                                ps[:],
                                lhsT=wo_sb[:, t, ot * P:(ot + 1) * P],
                                rhs=outt_sb[:, t, nsl],
                                start=(t == 0), stop=(t == OT - 1))
                        if ot % 2 == 0:
                            nc.vector.tensor_copy(fin[:, ot, :], ps[:])
                        else:
                            nc.scalar.copy(fin[:, ot, :], ps[:])
                        if (ot + 1) % half == 0:
                            lo = ot + 1 - half
                            nc.sync.dma_start(
                                yt[lo * P:(ot + 1) * P, nsl].rearrange(
                                    "(a p) s -> p a s", a=half),
                                fin[:, lo:ot + 1, :])
                    return go

                return [_one(ot) for ot in range(D // P)]

            def _wo_col(j, tail=False):
                nsl = slice(j * NQ, (j + 1) * NQ)
                half = D // P // 2
                fin = fin_sb.tile([P, D // P, NQ], BF16, name="fin")
                for ot in range(D // P):
                    if tail and ot % 2 == 1:
                        # attention is over: borrow the idle score pool so
                        # two output chunks are in flight
                        ps2 = score_ps.tile([P, 2, NQ], F32, name="wo_ps2",
                                            tag="sc")
                        ps = ps2[:, 0, :]
                    else:
                        ps = pp_ps.tile([P, NQ], F32, name="wo_ps", tag="pp")
                    for t in range(OT):
                        nc.tensor.matmul(
                            ps[:],
                            lhsT=wo_sb[:, t, ot * P:(ot + 1) * P],
                            rhs=outt_sb[:, t, nsl],
                            start=(t == 0), stop=(t == OT - 1))
                    # alternate engines so consecutive PSUM drains pipeline
                    if ot % 2 == 0:
                        nc.vector.tensor_copy(fin[:, ot, :], ps[:])
                    else:
                        nc.scalar.copy(fin[:, ot, :], ps[:])
                    step = 2 if tail else half
                    if (ot + 1) % step == 0:
                        # batched writeback: one DMA issue beats per-chunk
                        # 700ns-spaced issues; the tail uses quarters so the
                        # last transfer trails the last drain minimally
                        lo = ot + 1 - step
                        nc.sync.dma_start(
                            yt[lo * P:(ot + 1) * P, nsl].rearrange(
                                "(a p) s -> p a s", a=step),
                            fin[:, lo:ot + 1, :])

            for j in range(NT):
                # prefetch chunk j+1 while computing chunk j
                if j + 1 < NT:
                    jsl = slice((j + 1) * NQ, (j + 2) * NQ)
                    nc.sync.dma_start(
                        xt_sb[:, :, (j + 1) % 2, :],
                        xt[:, jsl].rearrange("(k p) s -> p k s", k=KT))
                    nc.sync.dma_start(cos_sb[:, jsl], cosr[:, jsl])
                    nc.sync.dma_start(sin_sb[:, jsl], sinr[:, jsl])
                if j == 0:
                    nc.sync.dma_start(wo_sb[:, :, :],
                                      wo[:, :, :].rearrange("t p d -> p t d"))
                # pair 0's q/k first: its rope chain gates the first scores;
                # pair 1's projection is deferred past attn0 so its rope
                # chain (DVE/Pool) doesn't queue ahead of attn0's masks
                _proj_qk(0, j)
                for st in range(4 * j, 4 * j + 4):
                    _proj_v(st)
                if j > 0:
                    _norm_b(1, j - 1, *saved)   # recips long done: no stall
                st0 = _attn(0, j, fillers=_proj_qk_fillers(1, j))
                if j > 0:
                    _wo_col(j - 1)   # fills PE while pair 0's recips run
                _norm_b(0, j, *st0)
                saved = _attn(1, j)
                if j == NT - 1:
                    tail_rcs = _recips(saved[0])
            _norm_b(1, NT - 1, *saved, tail=True, rcs=tail_rcs)
            _wo_col(NT - 1, tail=True)

    nc.compile()
    return nc


_NC_CACHE = {}


def _get_nc():
    if "nc" not in _NC_CACHE:
        _NC_CACHE["nc"] = build_nc()
    return _NC_CACHE["nc"]


_HALF = DK // 2
# 16-deep interleave: quadrant q holds freqs 16q..16q+15, x1 rows then x2 rows
_PERM = np.array([2 * (16 * q + i) + c
                  for q in (0, 1) for c in (0, 1) for i in range(16)])


BF16NP = ml_dtypes.bfloat16


def _prep_qk(W, heads):
    """Per-head RoPE-permuted projection weights (transposed for lhsT)."""
    Wh = W.reshape(H, DK, D)[heads][:, _PERM, :]                    # [NH, DK, D]
    return np.ascontiguousarray(Wh.reshape(NH * DK, D).T.astype(BF16NP))


def make_in_maps(x, token_positions, Wq, Wk, Wv, Wo):
    x = np.asarray(x, dtype=np.float32)
    Wq = np.asarray(Wq, dtype=np.float32)
    Wk = np.asarray(Wk, dtype=np.float32)
    Wv = np.asarray(Wv, dtype=np.float32)
    Wo = np.asarray(Wo, dtype=np.float32)
    pos = np.asarray(token_positions)

    j = np.arange(_HALF, dtype=np.float64)
    inv_freq = ROPE_THETA ** (-2.0 * j / DK)                        # [32]

    in_maps = []
    for core in range(NCORES):
        b = core // GROUPS
        g = core % GROUPS
        heads = list(range(g * NH, (g + 1) * NH))
        wq_ = _prep_qk(Wq, heads)
        wk_ = _prep_qk(Wk, heads)
        wv_ = np.ascontiguousarray(Wv.reshape(H, DK, D)[heads].reshape(NH * DK, D).T.astype(BF16NP))
        wo_ = np.ascontiguousarray(Wo.T.reshape(H, DK, D)[heads].reshape(OT, P, D).astype(BF16NP))
        ang = np.outer(inv_freq, pos[b].astype(np.float64))          # [32, S]
        cos32 = np.cos(ang)
        sin32 = np.sin(ang)
        # row layout per 64-block: [f0..15, f0..15, f16..31, f16..31]
        cos64 = np.concatenate([cos32[0:16], cos32[0:16],
                                cos32[16:32], cos32[16:32]])
        sin64 = np.concatenate([-sin32[0:16], sin32[0:16],
                                -sin32[16:32], sin32[16:32]])
        cosr = np.tile(cos64, (2, 1)).astype(np.float32)             # [128, S]
        sinr = np.tile(sin64, (2, 1)).astype(np.float32)
        in_maps.append({
            "xt": np.ascontiguousarray(x[b].T.astype(BF16NP)),
            "wq": wq_, "wk": wk_, "wv": wv_, "wo": wo_,
            "cosr": cosr, "sinr": sinr,
            "ones": np.ones((P, DK), dtype=np.float32),
        })
    return in_maps


def _gather(results):
    outs = [np.asarray(r["yt"], dtype=np.float32) for r in results]
    y = np.stack([
        sum(outs[b * GROUPS + 1: (b + 1) * GROUPS], outs[b * GROUPS]).T
        for b in range(B)
    ])
    return np.ascontiguousarray(y)


def kernel(x, token_positions, Wq, Wk, Wv, Wo):
    in_maps = make_in_maps(x, token_positions, Wq, Wk, Wv, Wo)
    res = run_bass_kernel_spmd(_get_nc(), in_maps, core_ids=list(range(NCORES)))
    return _gather(res.results)


def kernel_traced(x, token_positions, Wq, Wk, Wv, Wo, **kwargs):
    """Like kernel() but with NTFF tracing; returns (output, BassKernelResults)."""
    in_maps = make_in_maps(x, token_positions, Wq, Wk, Wv, Wo)
    res = run_bass_kernel_spmd(_get_nc(), in_maps, core_ids=list(range(NCORES)),
                               trace=True, **kwargs)
    return _gather(res.results), res


# revision 90
# speedup vs baseline: 1.0537x; 1.0040x over previous
"""Multi-head self-attention (RoPE, causal softmax) — Trainium2 Bass kernel.

Sharding over 8 NeuronCores: batch (2) x head-groups (16 heads / 4 groups).
Each core handles one batch element and 4 heads. Single software-pipelined
phase, streamed per query/key column block j (512 positions):

  - x / weights / cos/sin stream in as few batched multi-tile DMAs (the
    565ns-per-DMA descriptor-issue rate, not bytes, paces the prologue);
    x and the weights travel in bf16, halving prologue bytes; the first
    projection matmul starts ~3us in instead of ~36us
  - q/k projections for chunk j (head pairs stacked on partitions), RoPE
    via DVE stream_shuffle: head dims are host-permuted into a 16-deep
    interleave (x1/x2 component blocks of 16 rows inside each 32-partition
    quadrant) so the rotation partner is a within-quadrant 16-row swap —
    no second "rotated weights" matmul stream on the PE
  - v projection for key blocks 4j..4j+3 into v_aug (extra ones-column so
    the softmax denominator falls out of the attn@v matmul, PSUM row 64)
  - causal attention for column j, both head pairs: scoresT[m,n] blocks
    on PE writing a paired PSUM tile [128, 2, w], exp on ACT (scale=1/8
    fused) over both heads at once, causal mask via one gpsimd
    affine_select (pattern [[0,2],[1,w]]), attn@v per head; diagonal
    (masked) blocks are computed first so the end-of-loop drain consumes
    mask-free full blocks; scores run LAG blocks ahead of their attn@v
    consumers so the in-order PE never stalls on the exp->mask chain
  - per-head normalize, split in two stages: (A) DVE reciprocals of the
    denominator rows right after the i-loop, (B) ones-column PE broadcast
    matmul + drain + multiplies, deferred until fresh matmul work (next
    chunk's projections, wo) covers the latency; the odd head's rows reach
    partitions 64-127 via a gpsimd cross-partition copy (gpsimd may not
    touch PSUM, DVE may not cross partitions; this split satisfies both)
  - output projection wo(j-1) is emitted between attn pair 0 and pair 1 of
    chunk j; the final wo borrows the idle score-pool PSUM to run two
    output chunks in flight; partials are written to DRAM in bf16 via two
    batched half-column DMAs per chunk (host accumulates in fp32)

Attention matmuls run in float32r (full rate on the PE array); the x/W
projections run in bf16. Measured rel err ~3.7e-3 vs the fp32 reference.
"""

from contextlib import ExitStack

import ml_dtypes
import numpy as np

import concourse.bass as bass
import concourse.bacc as bacc
import concourse.tile as tile
from concourse import mybir
from concourse.bass_utils import run_bass_kernel_spmd

# problem shape (hardcoded: graded standalone)
B, S, D, H, DK = 2, 2048, 1024, 16, 64
NCORES = 8
GROUPS = NCORES // B  # 4 head-groups (cores) per batch element
NH = H // GROUPS      # 4 heads per core
ROPE_THETA = 10000.0

P = 128
NQ = 512              # query-block (matmul moving free dim)
NT = S // NQ          # 4 query blocks
NM = S // P           # 16 key blocks
KT = D // P           # 8 contraction tiles for the x-projections
OT = NH * DK // P     # 2 stacked head-pair tiles for q/k
LAG = 8               # score blocks emitted ahead of attn@v consumers

F32 = mybir.dt.float32
F32R = mybir.dt.float32r
BF16 = mybir.dt.bfloat16

# swap the two 16-row component blocks inside each 32-partition quadrant
SHUF_MASK = list(range(16, 32)) + list(range(0, 16))


def build_nc():
    nc = bacc.Bacc("TRN2", target_bir_lowering=False, debug=False)

    xt = nc.dram_tensor("xt", [D, S], BF16, kind="ExternalInput")        # x[b].T
    wq = nc.dram_tensor("wq", [D, NH * DK], BF16, kind="ExternalInput")  # perm'd, T
    wk = nc.dram_tensor("wk", [D, NH * DK], BF16, kind="ExternalInput")
    wv = nc.dram_tensor("wv", [D, NH * DK], BF16, kind="ExternalInput")
    wo = nc.dram_tensor("wo", [OT, P, D], BF16, kind="ExternalInput")   # pair-stacked
    cosr = nc.dram_tensor("cosr", [P, S], F32, kind="ExternalInput")
    sinr = nc.dram_tensor("sinr", [P, S], F32, kind="ExternalInput")    # sign-folded
    yt = nc.dram_tensor("yt", [D, S], BF16, kind="ExternalOutput")      # partial y.T
    ones_dram = nc.dram_tensor("ones", [P, DK], F32R, kind="ExternalInput")

    with tile.TileContext(nc) as tc, ExitStack() as ctx, \
            nc.allow_low_precision(reason="float32r matmul inputs and bf16 partial outputs are within tolerance"):
        # ---- persistent SBUF (whole kernel) ----
        persist = ctx.enter_context(tc.tile_pool(name="persist", bufs=1))
        cos_sb = persist.tile([P, S], F32, name="cos_sb")
        sin_sb = persist.tile([P, S], F32, name="sin_sb")
        ones_sb = persist.tile([P, DK], F32R, name="ones_sb")
        qtr_sb = persist.tile([P, OT, S], F32R, name="qtr_sb")    # rope(q)^T pairs
        ktr_sb = persist.tile([P, OT, S], F32R, name="ktr_sb")
        vaug_sb = persist.tile([P, NM, NH, DK + 1], F32R, name="vaug_sb")
        outt_sb = persist.tile([P, OT, S], BF16, name="outt_sb")  # out^T pairs
        wq_sb = persist.tile([P, KT, NH * DK], BF16, name="wq_sb")
        wk_sb = persist.tile([P, KT, NH * DK], BF16, name="wk_sb")
        wv_sb = persist.tile([P, KT, NH * DK], BF16, name="wv_sb")
        wo_sb = persist.tile([P, OT, D], BF16, name="wo_sb")
        xt_sb = persist.tile([P, KT, 2, NQ], BF16, name="xt_sb")  # j-chunk double buffer

        # prologue DMAs: batched multi-tile transfers — the SP queue's
        # 565ns/DMA issue rate, not bytes, paces the prologue, so one
        # 3-dim-AP DMA per tensor beats eight per-tile DMAs. xt rides the
        # scalar queue so both descriptor generators run in parallel.
        nc.sync.dma_start(wq_sb[:, 0:2, :],
                          wq[0:2 * P, :].rearrange("(k p) c -> p k c", k=2))
        nc.sync.dma_start(wq_sb[:, 2:KT // 2, :],
                          wq[2 * P:D // 2, :].rearrange("(k p) c -> p k c", k=2))
        nc.sync.dma_start(wq_sb[:, KT // 2:, :],
                          wq[D // 2:, :].rearrange("(k p) c -> p k c", k=KT // 2))
        nc.sync.dma_start(wk_sb[:, 0:KT // 2, :],
                          wk[0:D // 2, :].rearrange("(k p) c -> p k c", k=KT // 2))
        nc.sync.dma_start(wk_sb[:, KT // 2:, :],
                          wk[D // 2:, :].rearrange("(k p) c -> p k c", k=KT // 2))
        nc.scalar.dma_start(xt_sb[:, 0:2, 0, :],
                            xt[0:2 * P, 0:NQ].rearrange("(k p) s -> p k s", k=2))
        nc.scalar.dma_start(xt_sb[:, 2:KT // 2, 0, :],
                            xt[2 * P:D // 2, 0:NQ].rearrange("(k p) s -> p k s", k=2))
        nc.scalar.dma_start(xt_sb[:, KT // 2:, 0, :],
                            xt[D // 2:, 0:NQ].rearrange("(k p) s -> p k s", k=KT // 2))
        nc.sync.dma_start(cos_sb[:, 0:NQ], cosr[:, 0:NQ])
        nc.sync.dma_start(sin_sb[:, 0:NQ], sinr[:, 0:NQ])
        nc.sync.dma_start(wv_sb[:, :, :], wv[:, :].rearrange("(k p) c -> p k c", k=KT))
        # ones for the denominator broadcast; the scattered v_aug ones
        # column is filled by a cheap DVE copy (a strided DMA costs ~3.6us)
        nc.sync.dma_start(ones_sb[:], ones_dram[:, :])
        nc.vector.tensor_copy(vaug_sb[:, :, :, DK],
                              ones_sb[:].rearrange("p (a b) -> p a b", a=NM))

        pools = (
            tc.tile_pool(name="pp_ps", bufs=2, space="PSUM"),      # proj/v/bcast/wo
            tc.tile_pool(name="score_ps", bufs=2, space="PSUM"),   # paired scores
            tc.tile_pool(name="oaug_ps", bufs=2, space="PSUM"),
            tc.tile_pool(name="rope_sb", bufs=9),
            tc.tile_pool(name="exp_sb", bufs=11),
            tc.tile_pool(name="norm_sb", bufs=12),
            tc.tile_pool(name="fin_sb", bufs=2),
        )
        with pools[0] as pp_ps, pools[1] as score_ps, pools[2] as oaug_ps, \
                pools[3] as rope_sb, pools[4] as exp_sb, pools[5] as norm_sb, \
                pools[6] as fin_sb:

            def _proj_qk_fillers(t, j):
                """Pair-1 projection as a list of emission closures: the
                matmuls interleave into attn0's i-loop, filling the PE's
                exp-wait gaps (the i-loops run ~15% ACT-bound)."""
                csl = slice(j * NQ, (j + 1) * NQ)
                state = {}

                def _mm(qk, k, w_sb):
                    def go():
                        if qk not in state:
                            state[qk] = pp_ps.tile([P, NQ], F32,
                                                   name="projq_ps" if qk == 0
                                                   else "projk_ps", tag="pp")
                        nc.tensor.matmul(
                            state[qk][:],
                            lhsT=w_sb[:, k, t * P:(t + 1) * P],
                            rhs=xt_sb[:, k, j % 2, :],
                            start=(k == 0), stop=(k == KT - 1))
                    return go

                def _chain(qk, dst):
                    def go():
                        ps = state[qk][:]
                        shuf = rope_sb.tile([P, NQ], F32, name="shuf", tag="rope")
                        t1 = rope_sb.tile([P, NQ], F32, name="rope_a", tag="rope")
                        t2 = rope_sb.tile([P, NQ], F32, name="rope_b", tag="rope")
                        nc.vector.stream_shuffle(shuf[:], ps, SHUF_MASK)
                        nc.vector.tensor_mul(t1[:], ps, cos_sb[:, csl])
                        nc.gpsimd.tensor_mul(t2[:], shuf[:], sin_sb[:, csl])
                        nc.vector.tensor_add(dst[:, t, csl], t1[:], t2[:])
                    return go

                out = []
                for qk, w_sb, dst in ((0, wq_sb, qtr_sb), (1, wk_sb, ktr_sb)):
                    for k in range(KT):
                        out.append(_mm(qk, k, w_sb))
                    out.append(_chain(qk, dst))
                return out

            def _proj_qk(t, j):
                """q+k projection + rope for head pair t, chunk j.

                Pair 0 (emitted while the score pool is idle) interleaves the
                q/k k-loops in one paired score-pool tile — matching the
                prologue DMA triplet pacing. Pair 1 is emitted during attn0's
                score cycling, so it uses sequential k-loops on single-bank
                pp tiles instead of stealing a score buffer."""
                csl = slice(j * NQ, (j + 1) * NQ)
                if t == 0:
                    # sequential q-then-k loops in one paired score-pool tile
                    # (idle during projections): the q loop depends only on
                    # wq+xt, so the prologue feed keeps it PE-bound while wk
                    # streams in behind it
                    ps2 = score_ps.tile([P, 2, NQ], F32, name="proj_ps", tag="sc")
                    pss = (ps2[:, 0, :], ps2[:, 1, :])
                    for qk, w_sb in ((0, wq_sb), (1, wk_sb)):
                        for k in range(KT):
                            nc.tensor.matmul(
                                pss[qk],
                                lhsT=w_sb[:, k, t * P:(t + 1) * P],
                                rhs=xt_sb[:, k, j % 2, :],
                                start=(k == 0), stop=(k == KT - 1))
                else:
                    psq = pp_ps.tile([P, NQ], F32, name="projq_ps", tag="pp")
                    psk = pp_ps.tile([P, NQ], F32, name="projk_ps", tag="pp")
                    pss = (psq[:], psk[:])
                    for qk, w_sb in ((0, wq_sb), (1, wk_sb)):
                        for k in range(KT):
                            nc.tensor.matmul(
                                pss[qk],
                                lhsT=w_sb[:, k, t * P:(t + 1) * P],
                                rhs=xt_sb[:, k, j % 2, :],
                                start=(k == 0), stop=(k == KT - 1))
                for qk, dst in ((0, qtr_sb), (1, ktr_sb)):
                    ps = pss[qk]
                    shuf = rope_sb.tile([P, NQ], F32, name="shuf", tag="rope")
                    t1 = rope_sb.tile([P, NQ], F32, name="rope_a", tag="rope")
                    t2 = rope_sb.tile([P, NQ], F32, name="rope_b", tag="rope")
                    nc.vector.stream_shuffle(shuf[:], ps, SHUF_MASK)
                    nc.vector.tensor_mul(t1[:], ps, cos_sb[:, csl])
                    nc.gpsimd.tensor_mul(t2[:], shuf[:], sin_sb[:, csl])
                    nc.vector.tensor_add(dst[:, t, csl], t1[:], t2[:])

            def _proj_v(st, on_dve=False):
                psv = pp_ps.tile([P, NH * DK], F32, name="projv_ps", tag="pp")
                for k in range(KT):
                    nc.tensor.matmul(
                        psv[:],
                        lhsT=xt_sb[:, k, (st // 4) % 2, (st % 4) * P:(st % 4 + 1) * P],
                        rhs=wv_sb[:, k, :],
                        start=(k == 0), stop=(k == KT - 1))
                dst = vaug_sb[:, st, :, 0:DK]
                src = psv[:].rearrange("p (h c) -> p h c", h=NH)
                if on_dve:
                    nc.vector.tensor_copy(dst, src)
                else:
                    nc.scalar.copy(dst, src)

            def _attn(pr, j, fillers=None):
                """Causal attention for head pair pr, query chunk j."""
                imax = 4 * j + 3
                fillers = list(fillers or [])
                fidx = [0]

                def _fill(n):
                    while n > 0 and fidx[0] < len(fillers):
                        fillers[fidx[0]]()
                        fidx[0] += 1
                        n -= 1
                nsl = slice(j * NQ, (j + 1) * NQ)
                oa = [oaug_ps.tile([DK + 1, NQ], F32, name="oaug") for _ in range(2)]

                def _scores(i, start, stop):
                    # diagonal blocks (offset d = i - 4j): columns below
                    # 128*d are fully causal-masked — compute only [c0:512).
                    # Exception: d=3 would give N=128, which fp32r runs at a
                    # 4x penalty (ap_size < 256); widening it to N=256 halves
                    # its real cost — the extra columns are fully masked and
                    # contribute exact zeros to the attn@v accumulation
                    d = max(i - 4 * j, 0)
                    c0 = 2 * P if d == 3 else P * d
                    w = NQ - c0
                    sc = score_ps.tile([P, 2, NQ], F32, name="score", tag="sc")
                    for hh in range(2):
                        hb = hh * DK
                        nc.tensor.matmul(
                            sc[:, hh, 0:w],
                            lhsT=ktr_sb[hb:hb + DK, pr, i * P:(i + 1) * P],
                            rhs=qtr_sb[hb:hb + DK, pr,
                                       j * NQ + c0:(j + 1) * NQ],
                            start=True, stop=True)
                    eb = exp_sb.tile([P, 2, NQ], F32R, name="expblk")
                    nc.scalar.activation(
                        eb[:, :, 0:w], sc[:, :, 0:w],
                        mybir.ActivationFunctionType.Exp,
                        scale=float(1.0 / np.sqrt(DK)))
                    if i >= 4 * j:   # diagonal block: causal mask, both heads
                        # masked cells satisfy col - base < p <= 127: only
                        # the first 128+(-base) columns can ever be zeroed —
                        # restricting the AP there cuts the Pool op (and the
                        # chain the attn@v waits on); later columns are
                        # already correct in SBUF
                        base = -P if d == 3 else 0
                        mw = min(w, P - base)
                        nc.gpsimd.affine_select(
                            out=eb[:, :, 0:mw], in_=eb[:, :, 0:mw],
                            compare_op=mybir.AluOpType.is_ge,
                            fill=0.0,
                            base=base,
                            channel_multiplier=-1,
                            pattern=[[0, 2], [1, mw]])
                    return (eb, c0, w, start, stop)

                def _attnv(i, blk):
                    eb, c0, w, start, stop = blk
                    for hh in range(2):
                        nc.tensor.matmul(
                            oa[hh][:, c0:NQ],
                            lhsT=vaug_sb[:, i, 2 * pr + hh, :],
                            rhs=eb[:, hh, 0:w],
                            start=start, stop=stop)

                # diagonal blocks first: their exp->mask chains are the
                # longest, so the drain attn@v's at the end consume mask-free
                # full blocks instead of waiting on fresh masks
                i_order = list(range(4 * j, imax + 1)) + list(range(0, 4 * j))
                nsteps = imax + 1 + min(LAG, imax + 1)
                per_step = max(2, -(-len(fillers) // max(nsteps, 1)))
                pending = {}
                for pos, i in enumerate(i_order):
                    pending[i] = _scores(i, start=(pos == 0),
                                         stop=(pos == imax))
                    _fill(per_step)
                    if pos >= LAG:
                        ii = i_order[pos - LAG]
                        _attnv(ii, pending.pop(ii))
                for ii in i_order[max(0, imax + 1 - LAG):]:
                    _attnv(ii, pending.pop(ii))
                    _fill(per_step)
                _fill(len(fillers))   # flush any remainder

                return (oa,)

            def _recips(oa):
                rcs = []
                for hh in (1, 0):
                    rc = norm_sb.tile([DK + 1, NQ], F32R, name="recip", tag="nrm")
                    nc.vector.reciprocal(rc[DK:DK + 1, :], oa[hh][DK:DK + 1, :])
                    rcs.append((hh, rc))
                return rcs

            def _norm_b(pr, j, oa, tail=False, rcs=None):
                """Normalize: reciprocal of the denominator row (row 64, the
                ones column of v), broadcast, multiply into outt. Emitted
                behind filler matmul work so the PE never waits the chain."""
                nsl = slice(j * NQ, (j + 1) * NQ)
                if rcs is None:
                    rcs = _recips(oa)
                bss = []
                for hh, rc in rcs:
                    bc = pp_ps.tile([DK, NQ], F32, name="bcast_ps", tag="pp")
                    nc.tensor.matmul(
                        bc[:],
                        lhsT=ones_sb[DK:DK + 1, :],
                        rhs=rc[DK:DK + 1, :],
                        start=True, stop=True)
                    bs = norm_sb.tile([DK, NQ], F32, name="bcast_sb", tag="nrm")
                    if tail:
                        # final chain: ACT is idle, shorten the DVE span
                        nc.scalar.copy(bs[:], bc[:])
                    else:
                        nc.vector.tensor_copy(bs[:], bc[:])
                    bss.append((hh, bs))
                for hh, bs in bss:
                    if hh == 0:
                        nc.vector.tensor_mul(
                            outt_sb[0:DK, pr, nsl], oa[hh][0:DK, :], bs[:])
                    else:
                        # odd head: normalize at base 0 (DVE cannot cross
                        # partitions), then gpsimd places rows 64-127
                        ot_tmp = norm_sb.tile([DK, NQ], BF16, name="ot_tmp",
                                              tag="nrm")
                        nc.vector.tensor_mul(ot_tmp[:], oa[hh][0:DK, :], bs[:])
                        nc.gpsimd.tensor_copy(outt_sb[DK:P, pr, nsl], ot_tmp[:])

            def _wo_col_fillers(j):
                """wo(j) as emission closures interleaved into attn1(j+1)."""
                nsl = slice(j * NQ, (j + 1) * NQ)
                half = D // P // 2
                state = {}

                def _one(ot):
                    def go():
                        if 'fin' not in state:
                            state['fin'] = fin_sb.tile([P, D // P, NQ], BF16,
                                                       name="fin")
                        fin = state['fin']
                        ps = pp_ps.tile([P, NQ], F32, name="wo_ps", tag="pp")
                        for t in range(OT):
                            nc.tensor.matmul(
                                ps[:],
                                lhsT=wo_sb[:, t, ot * P:(ot + 1) * P],
                                rhs=outt_sb[:, t, nsl],
                                start=(t == 0), stop=(t == OT - 1))
                        if ot % 2 == 0:
                            nc.vector.tensor_copy(fin[:, ot, :], ps[:])
                        else:
                            nc.scalar.copy(fin[:, ot, :], ps[:])
                        if (ot + 1) % half == 0:
                            lo = ot + 1 - half
                            nc.sync.dma_start(
                                yt[lo * P:(ot + 1) * P, nsl].rearrange(
                                    "(a p) s -> p a s", a=half),
                                fin[:, lo:ot + 1, :])
                    return go

                return [_one(ot) for ot in range(D // P)]

            def _wo_col(j, tail=False):
                nsl = slice(j * NQ, (j + 1) * NQ)
                half = D // P // 2
                fin = fin_sb.tile([P, D // P, NQ], BF16, name="fin")
                for ot in range(D // P):
                    if tail and ot % 2 == 1:
                        # attention is over: borrow the idle score pool so
                        # two output chunks are in flight
                        ps2 = score_ps.tile([P, 2, NQ], F32, name="wo_ps2",
                                            tag="sc")
                        ps = ps2[:, 0, :]
                    else:
                        ps = pp_ps.tile([P, NQ], F32, name="wo_ps", tag="pp")
                    for t in range(OT):
                        nc.tensor.matmul(
                            ps[:],
                            lhsT=wo_sb[:, t, ot * P:(ot + 1) * P],
                            rhs=outt_sb[:, t, nsl],
                            start=(t == 0), stop=(t == OT - 1))
                    # alternate engines so consecutive PSUM drains pipeline
                    if ot % 2 == 0:
                        nc.vector.tensor_copy(fin[:, ot, :], ps[:])
                    else:
                        nc.scalar.copy(fin[:, ot, :], ps[:])
                    step = 2 if tail else half
                    if (ot + 1) % step == 0:
                        # batched writeback: one DMA issue beats per-chunk
                        # 700ns-spaced issues; the tail uses quarters so the
                        # last transfer trails the last drain minimally
                        lo = ot + 1 - step
                        nc.sync.dma_start(
                            yt[lo * P:(ot + 1) * P, nsl].rearrange(
                                "(a p) s -> p a s", a=step),
                            fin[:, lo:ot + 1, :])

            for j in range(NT):
                # prefetch chunk j+1 while computing chunk j
                if j + 1 < NT:
                    jsl = slice((j + 1) * NQ, (j + 2) * NQ)
                    nc.sync.dma_start(
                        xt_sb[:, :, (j + 1) % 2, :],
                        xt[:, jsl].rearrange("(k p) s -> p k s", k=KT))
                    nc.sync.dma_start(cos_sb[:, jsl], cosr[:, jsl])
                    nc.sync.dma_start(sin_sb[:, jsl], sinr[:, jsl])
                if j == 0:
                    nc.sync.dma_start(wo_sb[:, :, :],
                                      wo[:, :, :].rearrange("t p d -> p t d"))
                # pair 0's q/k first: its rope chain gates the first scores;
                # pair 1's projection is deferred past attn0 so its rope
                # chain (DVE/Pool) doesn't queue ahead of attn0's masks
                _proj_qk(0, j)
                for st in range(4 * j, 4 * j + 4):
                    _proj_v(st)
                if j > 0:
                    _norm_b(1, j - 1, *saved)   # recips long done: no stall
                st0 = _attn(0, j, fillers=_proj_qk_fillers(1, j))
                if j > 0:
                    _wo_col(j - 1)   # fills PE while pair 0's recips run
                _norm_b(0, j, *st0)
                saved = _attn(1, j)
                if j == NT - 1:
                    tail_rcs = _recips(saved[0])
            _norm_b(1, NT - 1, *saved, tail=True, rcs=tail_rcs)
            _wo_col(NT - 1, tail=True)

    nc.compile()
    return nc


_NC_CACHE = {}


def _get_nc():
    if "nc" not in _NC_CACHE:
        _NC_CACHE["nc"] = build_nc()
    return _NC_CACHE["nc"]


_HALF = DK // 2
# 16-deep interleave: quadrant q holds freqs 16q..16q+15, x1 rows then x2 rows
_PERM = np.array([2 * (16 * q + i) + c
                  for q in (0, 1) for c in (0, 1) for i in range(16)])


BF16NP = ml_dtypes.bfloat16


def _prep_qk(W, heads):
    """Per-head RoPE-permuted projection weights (transposed for lhsT)."""
    Wh = W.reshape(H, DK, D)[heads][:, _PERM, :]                    # [NH, DK, D]
    return np.ascontiguousarray(Wh.reshape(NH * DK, D).T.astype(BF16NP))


def make_in_maps(x, token_positions, Wq, Wk, Wv, Wo):
    x = np.asarray(x, dtype=np.float32)
    Wq = np.asarray(Wq, dtype=np.float32)
    Wk = np.asarray(Wk, dtype=np.float32)
    Wv = np.asarray(Wv, dtype=np.float32)
    Wo = np.asarray(Wo, dtype=np.float32)
    pos = np.asarray(token_positions)

    j = np.arange(_HALF, dtype=np.float64)
    inv_freq = ROPE_THETA ** (-2.0 * j / DK)                        # [32]

    in_maps = []
    for core in range(NCORES):
        b = core // GROUPS
        g = core % GROUPS
        heads = list(range(g * NH, (g + 1) * NH))
        wq_ = _prep_qk(Wq, heads)
        wk_ = _prep_qk(Wk, heads)
        wv_ = np.ascontiguousarray(Wv.reshape(H, DK, D)[heads].reshape(NH * DK, D).T.astype(BF16NP))
        wo_ = np.ascontiguousarray(Wo.T.reshape(H, DK, D)[heads].reshape(OT, P, D).astype(BF16NP))
        ang = np.outer(inv_freq, pos[b].astype(np.float64))          # [32, S]
        cos32 = np.cos(ang)
        sin32 = np.sin(ang)
        # row layout per 64-block: [f0..15, f0..15, f16..31, f16..31]
        cos64 = np.concatenate([cos32[0:16], cos32[0:16],
                                cos32[16:32], cos32[16:32]])
        sin64 = np.concatenate([-sin32[0:16], sin32[0:16],
                                -sin32[16:32], sin32[16:32]])
        cosr = np.tile(cos64, (2, 1)).astype(np.float32)             # [128, S]
        sinr = np.tile(sin64, (2, 1)).astype(np.float32)
        in_maps.append({
            "xt": np.ascontiguousarray(x[b].T.astype(BF16NP)),
            "wq": wq_, "wk": wk_, "wv": wv_, "wo": wo_,
            "cosr": cosr, "sinr": sinr,
            "ones": np.ones((P, DK), dtype=np.float32),
        })
    return in_maps


def _gather(results):
    outs = [np.asarray(r["yt"], dtype=np.float32) for r in results]
    y = np.stack([
        sum(outs[b * GROUPS + 1: (b + 1) * GROUPS], outs[b * GROUPS]).T
        for b in range(B)
    ])
    return np.ascontiguousarray(y)


def kernel(x, token_positions, Wq, Wk, Wv, Wo):
    in_maps = make_in_maps(x, token_positions, Wq, Wk, Wv, Wo)
    res = run_bass_kernel_spmd(_get_nc(), in_maps, core_ids=list(range(NCORES)))
    return _gather(res.results)


def kernel_traced(x, token_positions, Wq, Wk, Wv, Wo, **kwargs):
    """Like kernel() but with NTFF tracing; returns (output, BassKernelResults)."""
    in_maps = make_in_maps(x, token_positions, Wq, Wk, Wv, Wo)
    res = run_bass_kernel_spmd(_get_nc(), in_maps, core_ids=list(range(NCORES)),
                               trace=True, **kwargs)
    return _gather(res.results), res
